# revision 11
# baseline (speedup 1.0000x reference)
"""BN-LSTM (2-layer, Cooijmans) Trainium2 Bass kernel, 8-way batch-parallel.

Problem: B=2048, T=152, I=75, H=128, O=256. Training-mode BatchNorm over the
batch axis inside the recurrence => per-timestep cross-core statistics.

Strategy:
- Data-parallel over batch: 256 rows/core, params replicated.
- Layout: features on partitions, local batch on the free dim.
  h/c state tiles are (128=H, 256=B_loc); gate pre-acts are (128, 4, 256).
- Exact BN parity: per-step partial stats via vector.bn_stats, AllGathered
  across the 8 cores (DRAM bounce), combined with vector.bn_aggr.
- Input projections wi = x @ w_ih are computed in a bulk phase (stats shipped
  in ONE AllGather per layer); their BN is folded into the recurrence as
  pre_q = s_q*(wh_q + u'_q*wi_q) + v'_q with
    s_q   = gamma_hh/sd_hh          (per-step, from wh stats)
    u'_q  = (gamma_ih/gamma_hh) * sd_hh / sd_ih   (per-step scalar per feature)
    v'_q  = (beta_hh - m_hh*s) + (beta_ih + b - m_ih*u_ih)
  so each gate costs one vector scalar_tensor_tensor + one scalar activation.
"""

import time
from contextlib import ExitStack

import numpy as np

import concourse.bass as bass
import concourse.mybir as mybir
import concourse.bacc as bacc
import concourse.tile as tile
from concourse.bass_utils import run_bass_kernel_spmd

# ---- problem constants (hardcoded per harness contract) ----
B, T, I, H, O = 2048, 152, 75, 128, 256
NCORES = 8
BL = B // NCORES  # 256 local batch
G = 4             # gates f, i, o, g
EPS = 1e-5

fp32 = mybir.dt.float32
AF = mybir.ActivationFunctionType
ALU = mybir.AluOpType
RG = [list(range(NCORES))]


def _build():
    nc = bacc.Bacc("TRN2", target_bir_lowering=False, debug=False,
                   num_devices=NCORES)

    # ---- kernel I/O ----
    xT = nc.dram_tensor("xT", [I, T, BL], fp32, kind="ExternalInput").ap()
    wih0 = nc.dram_tensor("wih0", [I, G * H], fp32, kind="ExternalInput").ap()
    whh0 = nc.dram_tensor("whh0", [H, G * H], fp32, kind="ExternalInput").ap()
    wih1 = nc.dram_tensor("wih1", [H, G * H], fp32, kind="ExternalInput").ap()
    whh1 = nc.dram_tensor("whh1", [H, G * H], fp32, kind="ExternalInput").ap()
    fcwT = nc.dram_tensor("fcwT", [H, O], fp32, kind="ExternalInput").ap()
    pin = {}
    for l in range(2):
        for nm, sh in (("gih", [H, G]), ("beihb", [H, G]), ("ghh", [H, G]),
                       ("behh", [H, G]), ("gc", [H, 1]), ("bec", [H, 1])):
            key = f"{nm}{l}"
            pin[key] = nc.dram_tensor(key, sh, fp32, kind="ExternalInput").ap()
    y = nc.dram_tensor("y", [BL, O], fp32, kind="ExternalOutput").ap()

    with tile.TileContext(nc) as tc, ExitStack() as ctx:
        sb = ctx.enter_context(tc.tile_pool(name="sb", bufs=1))
        loop = ctx.enter_context(tc.tile_pool(name="loop", bufs=2))
        psum = ctx.enter_context(tc.tile_pool(name="psum", bufs=2, space="PSUM"))
        dram = ctx.enter_context(tc.tile_pool(name="dram", bufs=2, space="DRAM"))

        # ---- load params to SBUF ----
        def load(ap_in, shape, name):
            t_ = sb.tile(shape, fp32, name=name)
            nc.sync.dma_start(t_[:], ap_in[:])
            return t_

        wih0_sb = load(wih0, [I, G * H], "wih0_sb")
        whh0_sb = load(whh0, [H, G * H], "whh0_sb")
        wih1_sb = load(wih1, [H, G * H], "wih1_sb")
        whh1_sb = load(whh1, [H, G * H], "whh1_sb")
        fcwT_sb = load(fcwT, [H, O], "fcwT_sb")
        P = {}
        for l in range(2):
            for nm in ("gih", "beihb", "ghh", "behh"):
                key = f"{nm}{l}"
                P[key] = load(pin[key], [H, G], key + "_sb")
            for nm in ("gc", "bec"):
                key = f"{nm}{l}"
                P[key] = load(pin[key], [H, 1], key + "_sb")

        # per-layer gamma_ih/gamma_hh ratio
        ratio = {}
        for l in range(2):
            ig = sb.tile([H, G], fp32, name=f"invghh{l}")
            nc.vector.reciprocal(ig[:], P[f"ghh{l}"][:])
            r_ = sb.tile([H, G], fp32, name=f"ratio{l}")
            nc.vector.tensor_tensor(r_[:], P[f"gih{l}"][:], ig[:], op=ALU.mult)
            ratio[l] = r_

        # wi scratch in DRAM (per-core, internal)
        wi0_dram = dram.tile([T, H, G, BL], fp32, bufs=1, name="wi0_dram")
        wi1_dram = dram.tile([T, H, G, BL], fp32, bufs=1, name="wi1_dram")

        # per-(t,gate) wi-BN stats records, for the one-shot AllGather
        st_all = [sb.tile([H, T, G, 6], fp32, name=f"st{l}_all") for l in range(2)]
        # aggregated per-(t,gate) wi mean/var -> folded scales
        ugam_all = [sb.tile([H, T, G], fp32, name=f"ugam{l}") for l in range(2)]
        vih_all = [sb.tile([H, T, G], fp32, name=f"vih{l}") for l in range(2)]

        zeros_tg = sb.tile([H, T * G], fp32, name="zeros_tg")
        nc.vector.memset(zeros_tg[:], 0.0)
        eps_t = sb.tile([H, 1], fp32, name="eps_t")
        nc.vector.memset(eps_t[:], EPS)

        # ---------------- phase 0: wi0 = x @ w_ih0 (+stats) ----------------
        for t in range(T):
            xt = loop.tile([I, BL], fp32, tag="xt", bufs=3, name="xt")
            nc.sync.dma_start(xt[:], xT[:, t, :])
            pw = psum.tile([H, G, BL], fp32, tag="gp", name="pw")
            for q in range(G):
                nc.tensor.matmul(pw[:, q, :], wih0_sb[:, q * H:(q + 1) * H],
                                 xt[:], start=True, stop=True)
            for q in range(G):
                nc.vector.bn_stats(st_all[0][:, t, q, :], pw[:, q, :])
            wisb = loop.tile([H, G, BL], fp32, tag="wisb", bufs=3, name="wisb")
            nc.scalar.copy(wisb[:], pw[:])
            nc.sync.dma_start(wi0_dram[t], wisb[:])

        # ---- aggregate wi-layer stats across cores (one AG per layer) ----
        def aggregate_wi_stats(l):
            sbin = dram.tile([H, T * G * 6], fp32, tag=f"wistb{l}", name="sbin")
            sbout = dram.tile([NCORES * H, T * G * 6], fp32, tag=f"wistbo{l}",
                              addr_space="Shared", name="sbout")
            nc.sync.dma_start(sbin[:], st_all[l][:].rearrange("p t g s -> p (t g s)"))
            nc.gpsimd.collective_compute(
                "AllGather", ALU.bypass, replica_groups=RG,
                ins=[sbin[:]], outs=[sbout[:]],
            )
            mv_all = sb.tile([H, T, G, 2], fp32, tag="mv_all", name="mv_all")
            gview = sbout[:].rearrange("(r p) (t s) -> p r t s", r=NCORES, s=G * 6)
            CH = 19  # 152 = 8*19
            for c0 in range(0, T, CH):
                gst = loop.tile([H, NCORES, CH, G * 6], fp32, tag="gst",
                                name="gst")
                nc.sync.dma_start(gst[:], gview[:, :, c0:c0 + CH, :])
                for tl in range(CH):
                    for q in range(G):
                        nc.vector.bn_aggr(
                            mv_all[:, c0 + tl, q, :],
                            gst[:, :, tl, 6 * q:6 * (q + 1)])
            # broadcast params along t via activation-bias trick
            def bcast(src):  # src (H, G) -> (H, T, G)
                out = loop.tile([H, T, G], fp32, tag="bc", bufs=4, name="bc")
                for q in range(G):
                    nc.scalar.activation(out[:, :, q], zeros_tg[:, 0:T],
                                         AF.Identity, bias=src[:, q:q + 1])
                return out

            gih_bc = bcast(P[f"gih{l}"])
            ratio_bc = bcast(ratio[l])
            beihb_bc = bcast(P[f"beihb{l}"])
            mean_v = mv_all[:, :, :, 0]
            var_v = mv_all[:, :, :, 1]
            sd = loop.tile([H, T, G], fp32, tag="sd_all", name="sd")
            nc.scalar.activation(sd[:], var_v, AF.Sqrt, bias=eps_t[:])
            r_ = loop.tile([H, T, G], fp32, tag="r_all", name="r_")
            nc.vector.reciprocal(r_[:], sd[:])
            u_ = loop.tile([H, T, G], fp32, tag="u_all", name="u_")
            nc.vector.tensor_tensor(u_[:], r_[:], gih_bc[:], op=ALU.mult)
            nc.vector.tensor_tensor(ugam_all[l][:], r_[:], ratio_bc[:],
                                    op=ALU.mult)
            tmp = loop.tile([H, T, G], fp32, tag="tmp_all", name="tmp")
            nc.vector.tensor_tensor(tmp[:], mean_v, u_[:], op=ALU.mult)
            nc.vector.tensor_tensor(vih_all[l][:], beihb_bc[:], tmp[:],
                                    op=ALU.subtract)

        aggregate_wi_stats(0)

        # ---------------- recurrence ----------------
        def recurrence(l, whh_sb, wi_src_dram, produce_wi1):
            h = loop.tile([H, BL], fp32, tag=f"h{l}", name="h")
            c = loop.tile([H, BL], fp32, tag=f"c{l}", name="c")
            nc.vector.memset(h[:], 0.0)
            nc.vector.memset(c[:], 0.0)
            ghh, behh = P[f"ghh{l}"], P[f"behh{l}"]
            gc_, bec_ = P[f"gc{l}"], P[f"bec{l}"]
            for t in range(T):
                wi_t = loop.tile([H, G, BL], fp32, tag=f"wi_t{l}", bufs=3,
                                 name="wi_t")
                nc.sync.dma_start(wi_t[:], wi_src_dram[t])
                # wh = h @ w_hh  -> PSUM (H, G, BL)
                pw = psum.tile([H, G, BL], fp32, tag="gp", name="pwr")
                for q in range(G):
                    nc.tensor.matmul(pw[:, q, :], whh_sb[:, q * H:(q + 1) * H],
                                     h[:], start=True, stop=True)
                # wh stats -> AG -> aggr
                stw = loop.tile([H, G, 6], fp32, tag="stw", name="stw")
                for q in range(G):
                    nc.vector.bn_stats(stw[:, q, :], pw[:, q, :])
                gbin = dram.tile([H, G * 6], fp32, tag="gbin", name="gbin")
                gbout = dram.tile([NCORES * H, G * 6], fp32, tag="gbout",
                                  addr_space="Shared", name="gbout")
                nc.sync.dma_start(gbin[:], stw[:].rearrange("p g s -> p (g s)"))
                nc.gpsimd.collective_compute(
                    "AllGather", ALU.bypass, replica_groups=RG,
                    ins=[gbin[:]], outs=[gbout[:]],
                )
                gst8 = loop.tile([H, NCORES, G * 6], fp32, tag="gst8",
                                 name="gst8")
                nc.sync.dma_start(gst8[:],
                                  gbout[:].rearrange("(r p) s -> p r s",
                                                     r=NCORES))
                mv = loop.tile([H, G, 2], fp32, tag="mv", name="mv")
                for q in range(G):
                    nc.vector.bn_aggr(mv[:, q, :], gst8[:, :, 6 * q:6 * (q + 1)])
                # s = ghh / sqrt(v+eps); u' = ugam_t * sd; v' = behh - m*s + vih_t
                sd = loop.tile([H, G], fp32, tag="sd", name="sd")
                nc.scalar.activation(sd[:], mv[:, :, 1], AF.Sqrt, bias=eps_t[:])
                s_ = loop.tile([H, G], fp32, tag="s_", name="s_")
                rr = loop.tile([H, G], fp32, tag="rr", name="rr")
                nc.vector.reciprocal(rr[:], sd[:])
                nc.vector.tensor_tensor(s_[:], rr[:], ghh[:], op=ALU.mult)
                up = loop.tile([H, G], fp32, tag="up", name="up")
                nc.vector.tensor_tensor(up[:], ugam_all[l][:, t, :], sd[:],
                                        op=ALU.mult)
                vp = loop.tile([H, G], fp32, tag="vp", name="vp")
                tmg = loop.tile([H, G], fp32, tag="tmg", name="tmg")
                nc.vector.tensor_tensor(tmg[:], mv[:, :, 0], s_[:], op=ALU.mult)
                nc.vector.tensor_tensor(vp[:], behh[:], tmg[:], op=ALU.subtract)
                nc.vector.tensor_tensor(vp[:], vp[:], vih_all[l][:, t, :],
                                        op=ALU.add)
                # gates
                X = loop.tile([H, G, BL], fp32, tag="X", name="X")
                for q in range(G):
                    nc.vector.scalar_tensor_tensor(
                        X[:, q, :], in0=wi_t[:, q, :], scalar=up[:, q:q + 1],
                        in1=pw[:, q, :], op0=ALU.mult, op1=ALU.add)
                ga = loop.tile([H, G, BL], fp32, tag="ga", name="ga")
                for q, fn in enumerate((AF.Sigmoid, AF.Sigmoid, AF.Sigmoid,
                                        AF.Tanh)):
                    nc.scalar.activation(ga[:, q, :], X[:, q, :], fn,
                                         bias=vp[:, q:q + 1],
                                         scale=s_[:, q:q + 1])
                # c1 = f*c + i*g ; h1 = o * tanh(bn(c1))
                t1 = loop.tile([H, BL], fp32, tag="t1", name="t1")
                nc.vector.tensor_tensor(t1[:], ga[:, 1, :], ga[:, 3, :],
                                        op=ALU.mult)
                t2 = loop.tile([H, BL], fp32, tag="t2", name="t2")
                nc.vector.tensor_tensor(t2[:], ga[:, 0, :], c[:], op=ALU.mult)
                c = loop.tile([H, BL], fp32, tag=f"c{l}", name="c")
                nc.vector.tensor_tensor(c[:], t1[:], t2[:], op=ALU.add)
                stc = loop.tile([H, 6], fp32, tag="stc", name="stc")
                nc.vector.bn_stats(stc[:], c[:])
                cbin = dram.tile([H, 6], fp32, tag="cbin", name="cbin")
                cbout = dram.tile([NCORES * H, 6], fp32, tag="cbout",
                                  addr_space="Shared", name="cbout")
                nc.sync.dma_start(cbin[:], stc[:])
                nc.gpsimd.collective_compute(
                    "AllGather", ALU.bypass, replica_groups=RG,
                    ins=[cbin[:]], outs=[cbout[:]],
                )
                gstc = loop.tile([H, NCORES, 6], fp32, tag="gstc", name="gstc")
                nc.sync.dma_start(gstc[:],
                                  cbout[:].rearrange("(r p) s -> p r s",
                                                     r=NCORES))
                mvc = loop.tile([H, 2], fp32, tag="mvc", name="mvc")
                nc.vector.bn_aggr(mvc[:], gstc[:])
                sdc = loop.tile([H, 1], fp32, tag="sdc", name="sdc")
                nc.scalar.activation(sdc[:], mvc[:, 1:2], AF.Sqrt, bias=eps_t[:])
                rc = loop.tile([H, 1], fp32, tag="rc", name="rc")
                nc.vector.reciprocal(rc[:], sdc[:])
                sc = loop.tile([H, 1], fp32, tag="sc", name="sc")
                nc.vector.tensor_tensor(sc[:], rc[:], gc_[:], op=ALU.mult)
                tmc = loop.tile([H, 1], fp32, tag="tmc", name="tmc")
                nc.vector.tensor_tensor(tmc[:], mvc[:, 0:1], sc[:], op=ALU.mult)
                shc = loop.tile([H, 1], fp32, tag="shc", name="shc")
                nc.vector.tensor_tensor(shc[:], bec_[:], tmc[:],
                                        op=ALU.subtract)
                tnc = loop.tile([H, BL], fp32, tag="tnc", name="tnc")
                nc.scalar.activation(tnc[:], c[:], AF.Tanh, bias=shc[:],
                                     scale=sc[:])
                h = loop.tile([H, BL], fp32, tag=f"h{l}", name="h")
                nc.vector.tensor_tensor(h[:], ga[:, 2, :], tnc[:], op=ALU.mult)
                if produce_wi1:
                    pw1 = psum.tile([H, G, BL], fp32, tag="wi1p", name="pw1")
                    for q in range(G):
                        nc.tensor.matmul(pw1[:, q, :],
                                         wih1_sb[:, q * H:(q + 1) * H],
                                         h[:], start=True, stop=True)
                    for q in range(G):
                        nc.vector.bn_stats(st_all[1][:, t, q, :], pw1[:, q, :])
                    wo = loop.tile([H, G, BL], fp32, tag="wo", bufs=3,
                                   name="wo")
                    nc.scalar.copy(wo[:], pw1[:])
                    nc.sync.dma_start(wi1_dram[t], wo[:])
            return h

        recurrence(0, whh0_sb, wi0_dram, produce_wi1=True)
        aggregate_wi_stats(1)
        h_fin = recurrence(1, whh1_sb, wi1_dram, produce_wi1=False)

        # ---------------- final FC: y = h_fin.T @ fcwT ----------------
        for ci in range(2):
            pf = psum.tile([H, O], fp32, tag="wi1p", name="pf")
            nc.tensor.matmul(pf[:], h_fin[:, ci * H:(ci + 1) * H], fcwT_sb[:],
                             start=True, stop=True)
            yo = loop.tile([H, O], fp32, tag="yo", name="yo")
            nc.scalar.copy(yo[:], pf[:])
            nc.sync.dma_start(
                y[:].rearrange("(c p) o -> c p o", c=2)[ci], yo[:])

    nc.compile()
    return nc


_NC_CACHE = None


def _get_nc():
    global _NC_CACHE
    if _NC_CACHE is None:
        _NC_CACHE = _build()
    return _NC_CACHE


def _prep_inputs(sequences, w_ih0, w_hh0, b0, g_ih0, be_ih0, g_hh0, be_hh0,
                 g_c0, be_c0, w_ih1, w_hh1, b1, g_ih1, be_ih1, g_hh1, be_hh1,
                 g_c1, be_c1, fc_w, fc_b):
    f32 = np.float32

    def pg(v):  # (512,) -> (128, 4)
        return np.ascontiguousarray(np.asarray(v, f32).reshape(G, H).T)

    common = {
        "wih0": np.ascontiguousarray(np.asarray(w_ih0, f32)),
        "whh0": np.ascontiguousarray(np.asarray(w_hh0, f32)),
        "wih1": np.ascontiguousarray(np.asarray(w_ih1, f32)),
        "whh1": np.ascontiguousarray(np.asarray(w_hh1, f32)),
        "fcwT": np.ascontiguousarray(np.asarray(fc_w, f32).T),
        "gih0": pg(g_ih0), "beihb0": pg(np.asarray(be_ih0) + np.asarray(b0)),
        "ghh0": pg(g_hh0), "behh0": pg(be_hh0),
        "gc0": np.asarray(g_c0, f32).reshape(H, 1).copy(),
        "bec0": np.asarray(be_c0, f32).reshape(H, 1).copy(),
        "gih1": pg(g_ih1), "beihb1": pg(np.asarray(be_ih1) + np.asarray(b1)),
        "ghh1": pg(g_hh1), "behh1": pg(be_hh1),
        "gc1": np.asarray(g_c1, f32).reshape(H, 1).copy(),
        "bec1": np.asarray(be_c1, f32).reshape(H, 1).copy(),
    }
    seq = np.asarray(sequences, f32)
    in_maps = []
    for c in range(NCORES):
        m = dict(common)
        m["xT"] = np.ascontiguousarray(
            seq[c * BL:(c + 1) * BL].transpose(2, 1, 0))  # (I, T, BL)
        in_maps.append(m)
    return in_maps


def kernel(**inputs):
    nc = _get_nc()
    in_maps = _prep_inputs(**inputs)
    res = run_bass_kernel_spmd(nc, in_maps, core_ids=list(range(NCORES)),
                               trace=False)
    ys = [res.results[c]["y"] for c in range(NCORES)]
    out = np.concatenate(ys, axis=0)  # (B, O)
    out = out + np.asarray(inputs["fc_b"], np.float32)[None, :]
    return out.astype(np.float32)


# revision 21
# speedup vs baseline: 1.1093x; 1.1093x over previous
"""BN-LSTM (2-layer, Cooijmans) Trainium2 Bass kernel, 8-way batch-parallel.

Problem: B=2048, T=152, I=75, H=128, O=256. Training-mode BatchNorm over the
batch axis inside the recurrence => per-timestep cross-core statistics.

Strategy:
- Data-parallel over batch: 256 rows/core, params replicated.
- Layout: features on partitions, local batch on the free dim.
  h/c state tiles are (128=H, 256=B_loc); gate pre-acts are (128, 4, 256).
- Exact BN parity: per-step partial stats via vector.bn_stats, AllGathered
  across the 8 cores (DRAM bounce), combined with vector.bn_aggr.
- Input projections wi = x @ w_ih are computed in a bulk phase (stats shipped
  in ONE AllGather per layer); their BN is folded into the recurrence as
  pre_q = s_q*(wh_q + u'_q*wi_q) + v'_q with
    s_q   = gamma_hh/sd_hh          (per-step, from wh stats)
    u'_q  = (gamma_ih/gamma_hh) * sd_hh / sd_ih   (per-step scalar per feature)
    v'_q  = (beta_hh - m_hh*s) + (beta_ih + b - m_ih*u_ih)
  so each gate costs one vector scalar_tensor_tensor + one scalar activation.
"""

import time
from contextlib import ExitStack

import numpy as np

import concourse.bass as bass
import concourse.mybir as mybir
import concourse.bacc as bacc
import concourse.tile as tile
from concourse.bass_utils import run_bass_kernel_spmd

# ---- problem constants (hardcoded per harness contract) ----
B, T, I, H, O = 2048, 152, 75, 128, 256
NCORES = 8
BL = B // NCORES  # 256 local batch
G = 4             # gates f, i, o, g
EPS = 1e-5

fp32 = mybir.dt.float32
AF = mybir.ActivationFunctionType
ALU = mybir.AluOpType
RG = [list(range(NCORES))]


def _build(local_stats=False, repeats=1):
    """local_stats=True: numerically WRONG (per-shard BN) — timing probe only.
    repeats>1: run the whole pipeline N times serially (timing slope probe)."""
    nc = bacc.Bacc("TRN2", target_bir_lowering=False, debug=False,
                   num_devices=NCORES)

    # ---- kernel I/O ----
    xT = nc.dram_tensor("xT", [I, T, BL], fp32, kind="ExternalInput").ap()
    wih0 = nc.dram_tensor("wih0", [I, G * H], fp32, kind="ExternalInput").ap()
    whh0 = nc.dram_tensor("whh0", [H, G * H], fp32, kind="ExternalInput").ap()
    wih1 = nc.dram_tensor("wih1", [H, G * H], fp32, kind="ExternalInput").ap()
    whh1 = nc.dram_tensor("whh1", [H, G * H], fp32, kind="ExternalInput").ap()
    fcwT = nc.dram_tensor("fcwT", [H, O], fp32, kind="ExternalInput").ap()
    pin = {}
    for l in range(2):
        for nm, sh in (("gih", [H, G]), ("beihb", [H, G]), ("ghh", [H, G]),
                       ("behh", [H, G]), ("gc", [H, 1]), ("bec", [H, 1])):
            key = f"{nm}{l}"
            pin[key] = nc.dram_tensor(key, sh, fp32, kind="ExternalInput").ap()
    y = nc.dram_tensor("y", [BL, O], fp32, kind="ExternalOutput").ap()

    with tile.TileContext(nc) as tc, ExitStack() as ctx:
        sb = ctx.enter_context(tc.tile_pool(name="sb", bufs=1))
        loop = ctx.enter_context(tc.tile_pool(name="loop", bufs=2))
        psum = ctx.enter_context(tc.tile_pool(name="psum", bufs=2, space="PSUM"))
        dram = ctx.enter_context(tc.tile_pool(name="dram", bufs=2, space="DRAM"))

        # ---- load params to SBUF ----
        def load(ap_in, shape, name):
            t_ = sb.tile(shape, fp32, name=name)
            nc.sync.dma_start(t_[:], ap_in[:])
            return t_

        wih0_sb = load(wih0, [I, G * H], "wih0_sb")
        whh0_sb = load(whh0, [H, G * H], "whh0_sb")
        wih1_sb = load(wih1, [H, G * H], "wih1_sb")
        whh1_sb = load(whh1, [H, G * H], "whh1_sb")
        fcwT_sb = load(fcwT, [H, O], "fcwT_sb")
        P = {}
        for l in range(2):
            for nm in ("gih", "beihb", "ghh", "behh"):
                key = f"{nm}{l}"
                P[key] = load(pin[key], [H, G], key + "_sb")
            for nm in ("gc", "bec"):
                key = f"{nm}{l}"
                P[key] = load(pin[key], [H, 1], key + "_sb")

        # per-layer gamma_ih/gamma_hh ratio
        ratio = {}
        for l in range(2):
            ig = sb.tile([H, G], fp32, name=f"invghh{l}")
            nc.vector.reciprocal(ig[:], P[f"ghh{l}"][:])
            r_ = sb.tile([H, G], fp32, name=f"ratio{l}")
            nc.vector.tensor_tensor(r_[:], P[f"gih{l}"][:], ig[:], op=ALU.mult)
            ratio[l] = r_

        # wi scratch in DRAM (per-core, internal)
        wi0_dram = dram.tile([T, H, G, BL], fp32, bufs=1, name="wi0_dram")
        wi1_dram = dram.tile([T, H, G, BL], fp32, bufs=1, name="wi1_dram")

        # per-(t,gate) wi-BN stats records, for the one-shot AllGather
        st_all = [sb.tile([H, T, G, 6], fp32, name=f"st{l}_all") for l in range(2)]
        # aggregated per-(t,gate) wi mean/var -> folded scales
        ugam_all = [sb.tile([H, T, G], fp32, name=f"ugam{l}") for l in range(2)]
        vih_all = [sb.tile([H, T, G], fp32, name=f"vih{l}") for l in range(2)]

        zeros_tg = sb.tile([H, T * G], fp32, name="zeros_tg")
        nc.vector.memset(zeros_tg[:], 0.0)
        eps_t = sb.tile([H, 1], fp32, name="eps_t")
        nc.vector.memset(eps_t[:], EPS)

        # ---------------- phase 0: wi0 = x @ w_ih0 (+stats) ----------------
        def phase0():
          for t in range(T):
            xt = loop.tile([I, BL], fp32, tag="xt", bufs=3, name="xt")
            nc.sync.dma_start(xt[:], xT[:, t, :])
            pw = psum.tile([H, G, BL], fp32, tag="gp", name="pw")
            for q in range(G):
                nc.tensor.matmul(pw[:, q, :], wih0_sb[:, q * H:(q + 1) * H],
                                 xt[:], start=True, stop=True)
            for q in range(G):
                nc.vector.bn_stats(st_all[0][:, t, q, :], pw[:, q, :])
            wisb = loop.tile([H, G, BL], fp32, tag="wisb", bufs=3, name="wisb")
            nc.scalar.copy(wisb[:], pw[:])
            nc.sync.dma_start(wi0_dram[t], wisb[:])

        # ---- aggregate wi-layer stats across cores (one AG per layer) ----
        def aggregate_wi_stats(l):
            mv_all = sb.tile([H, T, G, 2], fp32, tag="mv_all", name="mv_all")
            if local_stats:
                for t_ in range(T):
                    for q in range(G):
                        nc.vector.bn_aggr(mv_all[:, t_, q, :],
                                          st_all[l][:, t_, q, :])
            else:
                sbin = dram.tile([H, T * G * 6], fp32, tag=f"wistb{l}",
                                 name="sbin")
                sbout = dram.tile([NCORES * H, T * G * 6], fp32,
                                  tag=f"wistbo{l}", addr_space="Shared",
                                  name="sbout")
                nc.sync.dma_start(
                    sbin[:], st_all[l][:].rearrange("p t g s -> p (t g s)"))
                nc.gpsimd.collective_compute(
                    "AllGather", ALU.bypass, replica_groups=RG,
                    ins=[sbin[:]], outs=[sbout[:]],
                )
                gview = sbout[:].rearrange("(r p) (t s) -> p r t s", r=NCORES,
                                           s=G * 6)
                CH = 19  # 152 = 8*19
                for c0 in range(0, T, CH):
                    gst = loop.tile([H, NCORES, CH, G * 6], fp32, tag="gst",
                                    name="gst")
                    nc.sync.dma_start(gst[:], gview[:, :, c0:c0 + CH, :])
                    for tl in range(CH):
                        for q in range(G):
                            nc.vector.bn_aggr(
                                mv_all[:, c0 + tl, q, :],
                                gst[:, :, tl, 6 * q:6 * (q + 1)])
            # broadcast params along t via activation-bias trick
            def bcast(src):  # src (H, G) -> (H, T, G)
                out = loop.tile([H, T, G], fp32, tag="bc", bufs=4, name="bc")
                for q in range(G):
                    nc.scalar.activation(out[:, :, q], zeros_tg[:, 0:T],
                                         AF.Identity, bias=src[:, q:q + 1])
                return out

            gih_bc = bcast(P[f"gih{l}"])
            ratio_bc = bcast(ratio[l])
            beihb_bc = bcast(P[f"beihb{l}"])
            mean_v = mv_all[:, :, :, 0]
            var_v = mv_all[:, :, :, 1]
            sd = loop.tile([H, T, G], fp32, tag="sd_all", name="sd")
            nc.scalar.activation(sd[:], var_v, AF.Sqrt, bias=eps_t[:])
            r_ = loop.tile([H, T, G], fp32, tag="r_all", name="r_")
            nc.vector.reciprocal(r_[:], sd[:])
            u_ = loop.tile([H, T, G], fp32, tag="u_all", name="u_")
            nc.vector.tensor_tensor(u_[:], r_[:], gih_bc[:], op=ALU.mult)
            nc.vector.tensor_tensor(ugam_all[l][:], r_[:], ratio_bc[:],
                                    op=ALU.mult)
            tmp = loop.tile([H, T, G], fp32, tag="tmp_all", name="tmp")
            nc.vector.tensor_tensor(tmp[:], mean_v, u_[:], op=ALU.mult)
            nc.vector.tensor_tensor(vih_all[l][:], beihb_bc[:], tmp[:],
                                    op=ALU.subtract)

        # ---------------- recurrence ----------------
        def recurrence(l, whh_sb, wi_src_dram, produce_wi1):
            h = loop.tile([H, BL], fp32, tag=f"h{l}", name="h")
            c = loop.tile([H, BL], fp32, tag=f"c{l}", name="c")
            nc.vector.memset(h[:], 0.0)
            nc.vector.memset(c[:], 0.0)
            ghh, behh = P[f"ghh{l}"], P[f"behh{l}"]
            gc_, bec_ = P[f"gc{l}"], P[f"bec{l}"]
            for t in range(T):
                wi_t = loop.tile([H, G, BL], fp32, tag=f"wi_t{l}", bufs=3,
                                 name="wi_t")
                nc.sync.dma_start(wi_t[:], wi_src_dram[t])
                # wh = h @ w_hh  -> PSUM (H, G, BL)
                pw = psum.tile([H, G, BL], fp32, tag="gp", name="pwr")
                for q in range(G):
                    nc.tensor.matmul(pw[:, q, :], whh_sb[:, q * H:(q + 1) * H],
                                     h[:], start=True, stop=True)
                # wh stats -> AG -> aggr
                stw = loop.tile([H, G, 6], fp32, tag="stw", name="stw")
                for q in range(G):
                    nc.vector.bn_stats(stw[:, q, :], pw[:, q, :])
                mv = loop.tile([H, G, 2], fp32, tag="mv", name="mv")
                if local_stats:
                    for q in range(G):
                        nc.vector.bn_aggr(mv[:, q, :], stw[:, q, :])
                else:
                    gbin = dram.tile([H, G * 6], fp32, tag="gbin", name="gbin")
                    gbout = dram.tile([NCORES * H, G * 6], fp32, tag="gbout",
                                      addr_space="Shared", name="gbout")
                    nc.sync.dma_start(gbin[:],
                                      stw[:].rearrange("p g s -> p (g s)"))
                    nc.gpsimd.collective_compute(
                        "AllGather", ALU.bypass, replica_groups=RG,
                        ins=[gbin[:]], outs=[gbout[:]],
                    )
                    gst8 = loop.tile([H, NCORES, G * 6], fp32, tag="gst8",
                                     name="gst8")
                    nc.sync.dma_start(gst8[:],
                                      gbout[:].rearrange("(r p) s -> p r s",
                                                         r=NCORES))
                    for q in range(G):
                        nc.vector.bn_aggr(mv[:, q, :],
                                          gst8[:, :, 6 * q:6 * (q + 1)])
                # s = ghh / sqrt(v+eps); u' = ugam_t * sd; v' = behh - m*s + vih_t
                sd = loop.tile([H, G], fp32, tag="sd", name="sd")
                nc.scalar.activation(sd[:], mv[:, :, 1], AF.Sqrt, bias=eps_t[:])
                s_ = loop.tile([H, G], fp32, tag="s_", name="s_")
                rr = loop.tile([H, G], fp32, tag="rr", name="rr")
                nc.vector.reciprocal(rr[:], sd[:])
                nc.vector.tensor_tensor(s_[:], rr[:], ghh[:], op=ALU.mult)
                up = loop.tile([H, G], fp32, tag="up", name="up")
                nc.vector.tensor_tensor(up[:], ugam_all[l][:, t, :], sd[:],
                                        op=ALU.mult)
                vp = loop.tile([H, G], fp32, tag="vp", name="vp")
                tmg = loop.tile([H, G], fp32, tag="tmg", name="tmg")
                nc.vector.tensor_tensor(tmg[:], mv[:, :, 0], s_[:], op=ALU.mult)
                nc.vector.tensor_tensor(vp[:], behh[:], tmg[:], op=ALU.subtract)
                nc.vector.tensor_tensor(vp[:], vp[:], vih_all[l][:, t, :],
                                        op=ALU.add)
                # gates
                X = loop.tile([H, G, BL], fp32, tag="X", name="X")
                for q in range(G):
                    nc.vector.scalar_tensor_tensor(
                        X[:, q, :], in0=wi_t[:, q, :], scalar=up[:, q:q + 1],
                        in1=pw[:, q, :], op0=ALU.mult, op1=ALU.add)
                ga = loop.tile([H, G, BL], fp32, tag="ga", name="ga")
                for q, fn in enumerate((AF.Sigmoid, AF.Sigmoid, AF.Sigmoid,
                                        AF.Tanh)):
                    nc.scalar.activation(ga[:, q, :], X[:, q, :], fn,
                                         bias=vp[:, q:q + 1],
                                         scale=s_[:, q:q + 1])
                # c1 = f*c + i*g ; h1 = o * tanh(bn(c1))
                t1 = loop.tile([H, BL], fp32, tag="t1", name="t1")
                nc.vector.tensor_tensor(t1[:], ga[:, 1, :], ga[:, 3, :],
                                        op=ALU.mult)
                t2 = loop.tile([H, BL], fp32, tag="t2", name="t2")
                nc.vector.tensor_tensor(t2[:], ga[:, 0, :], c[:], op=ALU.mult)
                c = loop.tile([H, BL], fp32, tag=f"c{l}", name="c")
                nc.vector.tensor_tensor(c[:], t1[:], t2[:], op=ALU.add)
                stc = loop.tile([H, 6], fp32, tag="stc", name="stc")
                nc.vector.bn_stats(stc[:], c[:])
                mvc = loop.tile([H, 2], fp32, tag="mvc", name="mvc")
                if local_stats:
                    nc.vector.bn_aggr(mvc[:], stc[:])
                else:
                    cbin = dram.tile([H, 6], fp32, tag="cbin", name="cbin")
                    cbout = dram.tile([NCORES * H, 6], fp32, tag="cbout",
                                      addr_space="Shared", name="cbout")
                    nc.sync.dma_start(cbin[:], stc[:])
                    nc.gpsimd.collective_compute(
                        "AllGather", ALU.bypass, replica_groups=RG,
                        ins=[cbin[:]], outs=[cbout[:]],
                    )
                    gstc = loop.tile([H, NCORES, 6], fp32, tag="gstc",
                                     name="gstc")
                    nc.sync.dma_start(gstc[:],
                                      cbout[:].rearrange("(r p) s -> p r s",
                                                         r=NCORES))
                    nc.vector.bn_aggr(mvc[:], gstc[:])
                sdc = loop.tile([H, 1], fp32, tag="sdc", name="sdc")
                nc.scalar.activation(sdc[:], mvc[:, 1:2], AF.Sqrt, bias=eps_t[:])
                rc = loop.tile([H, 1], fp32, tag="rc", name="rc")
                nc.vector.reciprocal(rc[:], sdc[:])
                sc = loop.tile([H, 1], fp32, tag="sc", name="sc")
                nc.vector.tensor_tensor(sc[:], rc[:], gc_[:], op=ALU.mult)
                tmc = loop.tile([H, 1], fp32, tag="tmc", name="tmc")
                nc.vector.tensor_tensor(tmc[:], mvc[:, 0:1], sc[:], op=ALU.mult)
                shc = loop.tile([H, 1], fp32, tag="shc", name="shc")
                nc.vector.tensor_tensor(shc[:], bec_[:], tmc[:],
                                        op=ALU.subtract)
                tnc = loop.tile([H, BL], fp32, tag="tnc", name="tnc")
                nc.scalar.activation(tnc[:], c[:], AF.Tanh, bias=shc[:],
                                     scale=sc[:])
                h = loop.tile([H, BL], fp32, tag=f"h{l}", name="h")
                nc.vector.tensor_tensor(h[:], ga[:, 2, :], tnc[:], op=ALU.mult)
                if produce_wi1:
                    pw1 = psum.tile([H, G, BL], fp32, tag="wi1p", name="pw1")
                    for q in range(G):
                        nc.tensor.matmul(pw1[:, q, :],
                                         wih1_sb[:, q * H:(q + 1) * H],
                                         h[:], start=True, stop=True)
                    for q in range(G):
                        nc.vector.bn_stats(st_all[1][:, t, q, :], pw1[:, q, :])
                    wo = loop.tile([H, G, BL], fp32, tag="wo", bufs=3,
                                   name="wo")
                    nc.scalar.copy(wo[:], pw1[:])
                    nc.sync.dma_start(wi1_dram[t], wo[:])
            return h

        for _rep in range(repeats):
            phase0()
            aggregate_wi_stats(0)
            recurrence(0, whh0_sb, wi0_dram, produce_wi1=True)
            aggregate_wi_stats(1)
            h_fin = recurrence(1, whh1_sb, wi1_dram, produce_wi1=False)

        # ---------------- final FC: y = h_fin.T @ fcwT ----------------
        for ci in range(2):
            pf = psum.tile([H, O], fp32, tag="wi1p", name="pf")
            nc.tensor.matmul(pf[:], h_fin[:, ci * H:(ci + 1) * H], fcwT_sb[:],
                             start=True, stop=True)
            yo = loop.tile([H, O], fp32, tag="yo", name="yo")
            nc.scalar.copy(yo[:], pf[:])
            nc.sync.dma_start(
                y[:].rearrange("(c p) o -> c p o", c=2)[ci], yo[:])

    nc.compile()
    return nc


def _build_v2(local_stats=False, repeats=1):
    """Interleaved two-layer pipeline: at step t, layer0 processes timestep t
    while layer1 processes timestep t-1. The per-step BN statistics of
    {wh0[t], wh1[t-1], wi1[t-1]} ride ONE AllGather, and {c0[t], c1[t-1]}
    ride a second — 2 collectives per step for both layers (vs 4)."""
    nc = bacc.Bacc("TRN2", target_bir_lowering=False, debug=False,
                   num_devices=NCORES)

    xT = nc.dram_tensor("xT", [I, T, BL], fp32, kind="ExternalInput").ap()
    wih0 = nc.dram_tensor("wih0", [I, G * H], fp32, kind="ExternalInput").ap()
    whh0 = nc.dram_tensor("whh0", [H, G * H], fp32, kind="ExternalInput").ap()
    wih1 = nc.dram_tensor("wih1", [H, G * H], fp32, kind="ExternalInput").ap()
    whh1 = nc.dram_tensor("whh1", [H, G * H], fp32, kind="ExternalInput").ap()
    fcwT = nc.dram_tensor("fcwT", [H, O], fp32, kind="ExternalInput").ap()
    pin = {}
    for nm, sh in (("gih0", [H, G]), ("beihb0", [H, G]), ("ratio0", [H, G]),
                   ("ratio1", [H, G]), ("gcat12", [H, 12]),
                   ("bcat12", [H, 12]), ("gc2", [H, 2]), ("bc2", [H, 2])):
        pin[nm] = nc.dram_tensor(nm, sh, fp32, kind="ExternalInput").ap()
    y = nc.dram_tensor("y", [BL, O], fp32, kind="ExternalOutput").ap()

    with tile.TileContext(nc) as tc, ExitStack() as ctx:
        sb = ctx.enter_context(tc.tile_pool(name="sb", bufs=1))
        loop = ctx.enter_context(tc.tile_pool(name="loop", bufs=2))
        psum = ctx.enter_context(tc.tile_pool(name="psum", bufs=1, space="PSUM"))
        dram = ctx.enter_context(tc.tile_pool(name="dram", bufs=2, space="DRAM"))

        def load(ap_in, shape, name):
            t_ = sb.tile(shape, fp32, name=name)
            nc.sync.dma_start(t_[:], ap_in[:])
            return t_

        wih0_sb = load(wih0, [I, G * H], "wih0_sb")
        whh0_sb = load(whh0, [H, G * H], "whh0_sb")
        wih1_sb = load(wih1, [H, G * H], "wih1_sb")
        whh1_sb = load(whh1, [H, G * H], "whh1_sb")
        fcwT_sb = load(fcwT, [H, O], "fcwT_sb")
        P = {nm: load(pin[nm], [H, {"gcat12": 12, "bcat12": 12,
                                    "gc2": 2, "bc2": 2}.get(nm, G)], nm + "_sb")
             for nm in pin}

        wi0_dram = dram.tile([T, H, G, BL], fp32, bufs=1, name="wi0_dram")
        st0_all = sb.tile([H, T, G, 6], fp32, name="st0_all")
        ugam0_all = sb.tile([H, T, G], fp32, name="ugam0_all")
        vih0_all = sb.tile([H, T, G], fp32, name="vih0_all")
        zeros_tg = sb.tile([H, T * G], fp32, name="zeros_tg")
        nc.vector.memset(zeros_tg[:], 0.0)
        eps_t = sb.tile([H, 1], fp32, name="eps_t")
        nc.vector.memset(eps_t[:], EPS)

        def phase0():
            for t in range(T):
                xt = loop.tile([I, BL], fp32, tag="xt", bufs=3, name="xt")
                nc.sync.dma_start(xt[:], xT[:, t, :])
                pw = psum.tile([H, G, BL], fp32, tag="g0", bufs=2, name="pw")
                for q in range(G):
                    nc.tensor.matmul(pw[:, q, :], wih0_sb[:, q * H:(q + 1) * H],
                                     xt[:], start=True, stop=True)
                for q in range(G):
                    nc.vector.bn_stats(st0_all[:, t, q, :], pw[:, q, :])
                wisb = loop.tile([H, G, BL], fp32, tag="wisb", bufs=3,
                                 name="wisb")
                nc.scalar.copy(wisb[:], pw[:])
                nc.sync.dma_start(wi0_dram[t], wisb[:])

        def aggregate0():
            mv_all = sb.tile([H, T, G, 2], fp32, tag="mv_all", name="mv_all")
            if local_stats:
                for t_ in range(T):
                    for q in range(G):
                        nc.vector.bn_aggr(mv_all[:, t_, q, :],
                                          st0_all[:, t_, q, :])
            else:
                sbin = dram.tile([H, T * G * 6], fp32, tag="wistb",
                                 name="sbin")
                sbout = dram.tile([NCORES * H, T * G * 6], fp32, tag="wistbo",
                                  addr_space="Shared", name="sbout")
                nc.sync.dma_start(
                    sbin[:], st0_all[:].rearrange("p t g s -> p (t g s)"))
                nc.gpsimd.collective_compute(
                    "AllGather", ALU.bypass, replica_groups=RG,
                    ins=[sbin[:]], outs=[sbout[:]],
                )
                gview = sbout[:].rearrange("(r p) (t s) -> p r t s", r=NCORES,
                                           s=G * 6)
                CH = 19
                for c0_ in range(0, T, CH):
                    gst = loop.tile([H, NCORES, CH, G * 6], fp32, tag="gst",
                                    name="gst")
                    nc.sync.dma_start(gst[:], gview[:, :, c0_:c0_ + CH, :])
                    for tl in range(CH):
                        for q in range(G):
                            nc.vector.bn_aggr(
                                mv_all[:, c0_ + tl, q, :],
                                gst[:, :, tl, 6 * q:6 * (q + 1)])

            def bcast(src):
                out = loop.tile([H, T, G], fp32, tag="bc", bufs=4, name="bc")
                for q in range(G):
                    nc.scalar.activation(out[:, :, q], zeros_tg[:, 0:T],
                                         AF.Identity, bias=src[:, q:q + 1])
                return out

            gih_bc = bcast(P["gih0"])
            ratio_bc = bcast(P["ratio0"])
            beihb_bc = bcast(P["beihb0"])
            mean_v = mv_all[:, :, :, 0]
            var_v = mv_all[:, :, :, 1]
            sd = loop.tile([H, T, G], fp32, tag="sd_all", name="sd")
            nc.scalar.activation(sd[:], var_v, AF.Sqrt, bias=eps_t[:])
            r_ = loop.tile([H, T, G], fp32, tag="r_all", name="r_")
            nc.vector.reciprocal(r_[:], sd[:])
            u_ = loop.tile([H, T, G], fp32, tag="u_all", name="u_")
            nc.vector.tensor_tensor(u_[:], r_[:], gih_bc[:], op=ALU.mult)
            nc.vector.tensor_tensor(ugam0_all[:], r_[:], ratio_bc[:],
                                    op=ALU.mult)
            tmp = loop.tile([H, T, G], fp32, tag="tmp_all", name="tmp")
            nc.vector.tensor_tensor(tmp[:], mean_v, u_[:], op=ALU.mult)
            nc.vector.tensor_tensor(vih0_all[:], beihb_bc[:], tmp[:],
                                    op=ALU.subtract)

        def mainloop():
            h0 = loop.tile([H, BL], fp32, tag="h0", name="h0")
            c0 = loop.tile([H, BL], fp32, tag="c0", name="c0")
            h1 = loop.tile([H, BL], fp32, tag="h1", name="h1")
            c1 = loop.tile([H, BL], fp32, tag="c1", name="c1")
            for t_ in (h0, c0, h1, c1):
                nc.vector.memset(t_[:], 0.0)
            wi1sb = None
            staga = loop.tile([H, 3, G, 6], fp32, tag="staga", bufs=2,
                              name="staga")
            nc.vector.memset(staga[:], 0.0)
            for t in range(T + 1):
                has0 = t < T
                has1 = t >= 1
                pw0 = pw1 = None
                if has0:
                    wi_t = loop.tile([H, G, BL], fp32, tag="wi0l", bufs=3,
                                     name="wi_t")
                    nc.sync.dma_start(wi_t[:], wi0_dram[t])
                    pw0 = psum.tile([H, G, BL], fp32, tag="g0", bufs=2,
                                    name="pw0")
                    for q in range(G):
                        nc.tensor.matmul(pw0[:, q, :],
                                         whh0_sb[:, q * H:(q + 1) * H],
                                         h0[:], start=True, stop=True)
                    for q in range(G):
                        nc.vector.bn_stats(staga[:, 0, q, :], pw0[:, q, :])
                if has1:
                    pw1 = psum.tile([H, G, BL], fp32, tag="g1", bufs=1,
                                    name="pw1")
                    for q in range(G):
                        nc.tensor.matmul(pw1[:, q, :],
                                         whh1_sb[:, q * H:(q + 1) * H],
                                         h1[:], start=True, stop=True)
                    for q in range(G):
                        nc.vector.bn_stats(staga[:, 1, q, :], pw1[:, q, :])
                # ---- AGa: {wh0, wh1, wi1} stats ----
                mv12 = loop.tile([H, 12, 2], fp32, tag="mv12", name="mv12")
                if local_stats:
                    for k in range(12):
                        nc.vector.bn_aggr(mv12[:, k, :],
                                          staga[:, k // 4, k % 4, :])
                else:
                    abin = dram.tile([H, 72], fp32, tag="abin", name="abin")
                    about = dram.tile([NCORES * H, 72], fp32, tag="about",
                                      addr_space="Shared", name="about")
                    nc.sync.dma_start(abin[:],
                                      staga[:].rearrange("p a g s -> p (a g s)"))
                    nc.gpsimd.collective_compute(
                        "AllGather", ALU.bypass, replica_groups=RG,
                        ins=[abin[:]], outs=[about[:]],
                    )
                    ag8 = loop.tile([H, NCORES, 72], fp32, tag="ag8",
                                    name="ag8")
                    nc.sync.dma_start(ag8[:],
                                      about[:].rearrange("(r p) s -> p r s",
                                                         r=NCORES))
                    for k in range(12):
                        nc.vector.bn_aggr(mv12[:, k, :],
                                          ag8[:, :, 6 * k:6 * (k + 1)])
                sd12 = loop.tile([H, 12], fp32, tag="sd12", name="sd12")
                nc.scalar.activation(sd12[:], mv12[:, :, 1], AF.Sqrt,
                                     bias=eps_t[:])
                r12 = loop.tile([H, 12], fp32, tag="r12", name="r12")
                nc.vector.reciprocal(r12[:], sd12[:])
                S12 = loop.tile([H, 12], fp32, tag="S12", name="S12")
                nc.vector.tensor_tensor(S12[:], r12[:], P["gcat12"][:],
                                        op=ALU.mult)
                TM12 = loop.tile([H, 12], fp32, tag="TM12", name="TM12")
                nc.vector.tensor_tensor(TM12[:], mv12[:, :, 0], S12[:],
                                        op=ALU.mult)
                SH12 = loop.tile([H, 12], fp32, tag="SH12", name="SH12")
                nc.vector.tensor_tensor(SH12[:], P["bcat12"][:], TM12[:],
                                        op=ALU.subtract)
                stagb = loop.tile([H, 2, 6], fp32, tag="stagb", bufs=2,
                                  name="stagb")
                if t == 0:
                    nc.vector.memset(stagb[:, 1, :], 0.0)
                if t == T:
                    nc.vector.memset(stagb[:, 0, :], 0.0)
                ga0 = ga1 = None
                c0n = c1n = None
                if has0:
                    u0p = loop.tile([H, G], fp32, tag="u0p", name="u0p")
                    nc.vector.tensor_tensor(u0p[:], ugam0_all[:, t, :],
                                            sd12[:, 0:4], op=ALU.mult)
                    v0p = loop.tile([H, G], fp32, tag="v0p", name="v0p")
                    nc.vector.tensor_tensor(v0p[:], SH12[:, 0:4],
                                            vih0_all[:, t, :], op=ALU.add)
                    for q in range(G):
                        nc.vector.scalar_tensor_tensor(
                            pw0[:, q, :], in0=wi_t[:, q, :],
                            scalar=u0p[:, q:q + 1], in1=pw0[:, q, :],
                            op0=ALU.mult, op1=ALU.add)
                    ga0 = loop.tile([H, G, BL], fp32, tag="ga0", bufs=2,
                                    name="ga0")
                    for q, fn in enumerate((AF.Sigmoid, AF.Sigmoid,
                                            AF.Sigmoid, AF.Tanh)):
                        nc.scalar.activation(ga0[:, q, :], pw0[:, q, :], fn,
                                             bias=v0p[:, q:q + 1],
                                             scale=S12[:, q:q + 1])
                    t10 = loop.tile([H, BL], fp32, tag="t10", name="t10")
                    nc.vector.tensor_tensor(t10[:], ga0[:, 1, :], ga0[:, 3, :],
                                            op=ALU.mult)
                    t20 = loop.tile([H, BL], fp32, tag="t20", name="t20")
                    nc.vector.tensor_tensor(t20[:], ga0[:, 0, :], c0[:],
                                            op=ALU.mult)
                    c0n = loop.tile([H, BL], fp32, tag="c0", name="c0n")
                    nc.vector.tensor_tensor(c0n[:], t10[:], t20[:], op=ALU.add)
                    nc.vector.bn_stats(stagb[:, 0, :], c0n[:])
                if has1:
                    tu1 = loop.tile([H, G], fp32, tag="tu1", name="tu1")
                    nc.vector.tensor_tensor(tu1[:], P["ratio1"][:],
                                            r12[:, 8:12], op=ALU.mult)
                    u1p = loop.tile([H, G], fp32, tag="u1p", name="u1p")
                    nc.vector.tensor_tensor(u1p[:], tu1[:], sd12[:, 4:8],
                                            op=ALU.mult)
                    v1p = loop.tile([H, G], fp32, tag="v1p", name="v1p")
                    nc.vector.tensor_tensor(v1p[:], SH12[:, 4:8], SH12[:, 8:12],
                                            op=ALU.add)
                    for q in range(G):
                        nc.vector.scalar_tensor_tensor(
                            pw1[:, q, :], in0=wi1sb[:, q, :],
                            scalar=u1p[:, q:q + 1], in1=pw1[:, q, :],
                            op0=ALU.mult, op1=ALU.add)
                    ga1 = loop.tile([H, G, BL], fp32, tag="ga1", bufs=2,
                                    name="ga1")
                    for q, fn in enumerate((AF.Sigmoid, AF.Sigmoid,
                                            AF.Sigmoid, AF.Tanh)):
                        nc.scalar.activation(ga1[:, q, :], pw1[:, q, :], fn,
                                             bias=v1p[:, q:q + 1],
                                             scale=S12[:, 4 + q:5 + q])
                    t11 = loop.tile([H, BL], fp32, tag="t11", name="t11")
                    nc.vector.tensor_tensor(t11[:], ga1[:, 1, :], ga1[:, 3, :],
                                            op=ALU.mult)
                    t21 = loop.tile([H, BL], fp32, tag="t21", name="t21")
                    nc.vector.tensor_tensor(t21[:], ga1[:, 0, :], c1[:],
                                            op=ALU.mult)
                    c1n = loop.tile([H, BL], fp32, tag="c1", name="c1n")
                    nc.vector.tensor_tensor(c1n[:], t11[:], t21[:], op=ALU.add)
                    nc.vector.bn_stats(stagb[:, 1, :], c1n[:])
                # ---- AGb: {c0, c1} stats ----
                mvc2 = loop.tile([H, 2, 2], fp32, tag="mvc2", name="mvc2")
                if local_stats:
                    for k in range(2):
                        nc.vector.bn_aggr(mvc2[:, k, :], stagb[:, k, :])
                else:
                    bbin = dram.tile([H, 12], fp32, tag="bbin", name="bbin")
                    bbout = dram.tile([NCORES * H, 12], fp32, tag="bbout",
                                      addr_space="Shared", name="bbout")
                    nc.sync.dma_start(bbin[:],
                                      stagb[:].rearrange("p a s -> p (a s)"))
                    nc.gpsimd.collective_compute(
                        "AllGather", ALU.bypass, replica_groups=RG,
                        ins=[bbin[:]], outs=[bbout[:]],
                    )
                    bg8 = loop.tile([H, NCORES, 12], fp32, tag="bg8",
                                    name="bg8")
                    nc.sync.dma_start(bg8[:],
                                      bbout[:].rearrange("(r p) s -> p r s",
                                                         r=NCORES))
                    for k in range(2):
                        nc.vector.bn_aggr(mvc2[:, k, :],
                                          bg8[:, :, 6 * k:6 * (k + 1)])
                sdc2 = loop.tile([H, 2], fp32, tag="sdc2", name="sdc2")
                nc.scalar.activation(sdc2[:], mvc2[:, :, 1], AF.Sqrt,
                                     bias=eps_t[:])
                rc2 = loop.tile([H, 2], fp32, tag="rc2", name="rc2")
                nc.vector.reciprocal(rc2[:], sdc2[:])
                scc = loop.tile([H, 2], fp32, tag="scc", name="scc")
                nc.vector.tensor_tensor(scc[:], rc2[:], P["gc2"][:],
                                        op=ALU.mult)
                tmc = loop.tile([H, 2], fp32, tag="tmc", name="tmc")
                nc.vector.tensor_tensor(tmc[:], mvc2[:, :, 0], scc[:],
                                        op=ALU.mult)
                shc = loop.tile([H, 2], fp32, tag="shc", name="shc")
                nc.vector.tensor_tensor(shc[:], P["bc2"][:], tmc[:],
                                        op=ALU.subtract)
                if has0:
                    tn0 = loop.tile([H, BL], fp32, tag="tn0", name="tn0")
                    nc.scalar.activation(tn0[:], c0n[:], AF.Tanh,
                                         bias=shc[:, 0:1], scale=scc[:, 0:1])
                    h0n = loop.tile([H, BL], fp32, tag="h0", name="h0n")
                    nc.vector.tensor_tensor(h0n[:], ga0[:, 2, :], tn0[:],
                                            op=ALU.mult)
                    # produce wi1[t] for layer 1 (consumed next step)
                    pwm = psum.tile([H, G, BL], fp32, tag="w1", bufs=1,
                                    name="pwm")
                    for q in range(G):
                        nc.tensor.matmul(pwm[:, q, :],
                                         wih1_sb[:, q * H:(q + 1) * H],
                                         h0n[:], start=True, stop=True)
                    staga_n = loop.tile([H, 3, G, 6], fp32, tag="staga",
                                        bufs=2, name="staga_n")
                    for q in range(G):
                        nc.vector.bn_stats(staga_n[:, 2, q, :], pwm[:, q, :])
                    wi1n = loop.tile([H, G, BL], fp32, tag="wi1sb", bufs=2,
                                     name="wi1n")
                    nc.scalar.copy(wi1n[:], pwm[:])
                    wi1sb = wi1n
                    staga = staga_n
                    h0 = h0n
                    c0 = c0n
                if has1:
                    tn1 = loop.tile([H, BL], fp32, tag="tn1", name="tn1")
                    nc.scalar.activation(tn1[:], c1n[:], AF.Tanh,
                                         bias=shc[:, 1:2], scale=scc[:, 1:2])
                    h1n = loop.tile([H, BL], fp32, tag="h1", name="h1n")
                    nc.vector.tensor_tensor(h1n[:], ga1[:, 2, :], tn1[:],
                                            op=ALU.mult)
                    h1 = h1n
                    c1 = c1n
            return h1

        for _rep in range(repeats):
            phase0()
            aggregate0()
            h_fin = mainloop()

        for ci in range(2):
            pf = psum.tile([H, O], fp32, tag="w1", name="pf")
            nc.tensor.matmul(pf[:], h_fin[:, ci * H:(ci + 1) * H], fcwT_sb[:],
                             start=True, stop=True)
            yo = loop.tile([H, O], fp32, tag="yo", name="yo")
            nc.scalar.copy(yo[:], pf[:])
            nc.sync.dma_start(
                y[:].rearrange("(c p) o -> c p o", c=2)[ci], yo[:])

    nc.compile()
    return nc


VERSION = 2

_NC_CACHE = None


def _get_nc():
    global _NC_CACHE
    if _NC_CACHE is None:
        _NC_CACHE = _build_v2() if VERSION == 2 else _build()
    return _NC_CACHE


def _prep_inputs(sequences, w_ih0, w_hh0, b0, g_ih0, be_ih0, g_hh0, be_hh0,
                 g_c0, be_c0, w_ih1, w_hh1, b1, g_ih1, be_ih1, g_hh1, be_hh1,
                 g_c1, be_c1, fc_w, fc_b):
    f32 = np.float32

    def pg(v):  # (512,) -> (128, 4)
        return np.ascontiguousarray(np.asarray(v, f32).reshape(G, H).T)

    common = {
        "wih0": np.ascontiguousarray(np.asarray(w_ih0, f32)),
        "whh0": np.ascontiguousarray(np.asarray(w_hh0, f32)),
        "wih1": np.ascontiguousarray(np.asarray(w_ih1, f32)),
        "whh1": np.ascontiguousarray(np.asarray(w_hh1, f32)),
        "fcwT": np.ascontiguousarray(np.asarray(fc_w, f32).T),
    }
    if VERSION == 2:
        common.update({
            "gih0": pg(g_ih0),
            "beihb0": pg(np.asarray(be_ih0) + np.asarray(b0)),
            "ratio0": pg(np.asarray(g_ih0) / np.asarray(g_hh0)),
            "ratio1": pg(np.asarray(g_ih1) / np.asarray(g_hh1)),
            "gcat12": np.concatenate([pg(g_hh0), pg(g_hh1), pg(g_ih1)],
                                     axis=1),
            "bcat12": np.concatenate(
                [pg(be_hh0), pg(be_hh1),
                 pg(np.asarray(be_ih1) + np.asarray(b1))], axis=1),
            "gc2": np.stack([np.asarray(g_c0, f32),
                             np.asarray(g_c1, f32)], axis=1).copy(),
            "bc2": np.stack([np.asarray(be_c0, f32),
                             np.asarray(be_c1, f32)], axis=1).copy(),
        })
    else:
        common.update({
            "gih0": pg(g_ih0),
            "beihb0": pg(np.asarray(be_ih0) + np.asarray(b0)),
            "ghh0": pg(g_hh0), "behh0": pg(be_hh0),
            "gc0": np.asarray(g_c0, f32).reshape(H, 1).copy(),
            "bec0": np.asarray(be_c0, f32).reshape(H, 1).copy(),
            "gih1": pg(g_ih1),
            "beihb1": pg(np.asarray(be_ih1) + np.asarray(b1)),
            "ghh1": pg(g_hh1), "behh1": pg(be_hh1),
            "gc1": np.asarray(g_c1, f32).reshape(H, 1).copy(),
            "bec1": np.asarray(be_c1, f32).reshape(H, 1).copy(),
        })
    seq = np.asarray(sequences, f32)
    in_maps = []
    for c in range(NCORES):
        m = dict(common)
        m["xT"] = np.ascontiguousarray(
            seq[c * BL:(c + 1) * BL].transpose(2, 1, 0))  # (I, T, BL)
        in_maps.append(m)
    return in_maps


def kernel(**inputs):
    nc = _get_nc()
    in_maps = _prep_inputs(**inputs)
    res = run_bass_kernel_spmd(nc, in_maps, core_ids=list(range(NCORES)),
                               trace=False)
    ys = [res.results[c]["y"] for c in range(NCORES)]
    out = np.concatenate(ys, axis=0)  # (B, O)
    out = out + np.asarray(inputs["fc_b"], np.float32)[None, :]
    return out.astype(np.float32)


# revision 26
# speedup vs baseline: 1.1373x; 1.0252x over previous
"""BN-LSTM (2-layer, Cooijmans) Trainium2 Bass kernel, 8-way batch-parallel.

Problem: B=2048, T=152, I=75, H=128, O=256. Training-mode BatchNorm over the
batch axis inside the recurrence => per-timestep cross-core statistics.

Strategy:
- Data-parallel over batch: 256 rows/core, params replicated.
- Layout: features on partitions, local batch on the free dim.
  h/c state tiles are (128=H, 256=B_loc); gate pre-acts are (128, 4, 256).
- Exact BN parity: per-step partial stats via vector.bn_stats, AllGathered
  across the 8 cores (DRAM bounce), combined with vector.bn_aggr.
- Input projections wi = x @ w_ih are computed in a bulk phase (stats shipped
  in ONE AllGather per layer); their BN is folded into the recurrence as
  pre_q = s_q*(wh_q + u'_q*wi_q) + v'_q with
    s_q   = gamma_hh/sd_hh          (per-step, from wh stats)
    u'_q  = (gamma_ih/gamma_hh) * sd_hh / sd_ih   (per-step scalar per feature)
    v'_q  = (beta_hh - m_hh*s) + (beta_ih + b - m_ih*u_ih)
  so each gate costs one vector scalar_tensor_tensor + one scalar activation.
"""

import time
from contextlib import ExitStack

import numpy as np

import concourse.bass as bass
import concourse.mybir as mybir
import concourse.bacc as bacc
import concourse.tile as tile
from concourse.bass_utils import run_bass_kernel_spmd

# ---- problem constants (hardcoded per harness contract) ----
B, T, I, H, O = 2048, 152, 75, 128, 256
NCORES = 8
BL = B // NCORES  # 256 local batch
G = 4             # gates f, i, o, g
EPS = 1e-5

fp32 = mybir.dt.float32
AF = mybir.ActivationFunctionType
ALU = mybir.AluOpType
RG = [list(range(NCORES))]


def _build(local_stats=False, repeats=1):
    """local_stats=True: numerically WRONG (per-shard BN) — timing probe only.
    repeats>1: run the whole pipeline N times serially (timing slope probe)."""
    nc = bacc.Bacc("TRN2", target_bir_lowering=False, debug=False,
                   num_devices=NCORES)

    # ---- kernel I/O ----
    xT = nc.dram_tensor("xT", [I, T, BL], fp32, kind="ExternalInput").ap()
    wih0 = nc.dram_tensor("wih0", [I, G * H], fp32, kind="ExternalInput").ap()
    whh0 = nc.dram_tensor("whh0", [H, G * H], fp32, kind="ExternalInput").ap()
    wih1 = nc.dram_tensor("wih1", [H, G * H], fp32, kind="ExternalInput").ap()
    whh1 = nc.dram_tensor("whh1", [H, G * H], fp32, kind="ExternalInput").ap()
    fcwT = nc.dram_tensor("fcwT", [H, O], fp32, kind="ExternalInput").ap()
    pin = {}
    for l in range(2):
        for nm, sh in (("gih", [H, G]), ("beihb", [H, G]), ("ghh", [H, G]),
                       ("behh", [H, G]), ("gc", [H, 1]), ("bec", [H, 1])):
            key = f"{nm}{l}"
            pin[key] = nc.dram_tensor(key, sh, fp32, kind="ExternalInput").ap()
    y = nc.dram_tensor("y", [BL, O], fp32, kind="ExternalOutput").ap()

    with tile.TileContext(nc) as tc, ExitStack() as ctx:
        sb = ctx.enter_context(tc.tile_pool(name="sb", bufs=1))
        loop = ctx.enter_context(tc.tile_pool(name="loop", bufs=2))
        psum = ctx.enter_context(tc.tile_pool(name="psum", bufs=2, space="PSUM"))
        dram = ctx.enter_context(tc.tile_pool(name="dram", bufs=2, space="DRAM"))

        # ---- load params to SBUF ----
        def load(ap_in, shape, name):
            t_ = sb.tile(shape, fp32, name=name)
            nc.sync.dma_start(t_[:], ap_in[:])
            return t_

        wih0_sb = load(wih0, [I, G * H], "wih0_sb")
        whh0_sb = load(whh0, [H, G * H], "whh0_sb")
        wih1_sb = load(wih1, [H, G * H], "wih1_sb")
        whh1_sb = load(whh1, [H, G * H], "whh1_sb")
        fcwT_sb = load(fcwT, [H, O], "fcwT_sb")
        P = {}
        for l in range(2):
            for nm in ("gih", "beihb", "ghh", "behh"):
                key = f"{nm}{l}"
                P[key] = load(pin[key], [H, G], key + "_sb")
            for nm in ("gc", "bec"):
                key = f"{nm}{l}"
                P[key] = load(pin[key], [H, 1], key + "_sb")

        # per-layer gamma_ih/gamma_hh ratio
        ratio = {}
        for l in range(2):
            ig = sb.tile([H, G], fp32, name=f"invghh{l}")
            nc.vector.reciprocal(ig[:], P[f"ghh{l}"][:])
            r_ = sb.tile([H, G], fp32, name=f"ratio{l}")
            nc.vector.tensor_tensor(r_[:], P[f"gih{l}"][:], ig[:], op=ALU.mult)
            ratio[l] = r_

        # wi scratch in DRAM (per-core, internal)
        wi0_dram = dram.tile([T, H, G, BL], fp32, bufs=1, name="wi0_dram")
        wi1_dram = dram.tile([T, H, G, BL], fp32, bufs=1, name="wi1_dram")

        # per-(t,gate) wi-BN stats records, for the one-shot AllGather
        st_all = [sb.tile([H, T, G, 6], fp32, name=f"st{l}_all") for l in range(2)]
        # aggregated per-(t,gate) wi mean/var -> folded scales
        ugam_all = [sb.tile([H, T, G], fp32, name=f"ugam{l}") for l in range(2)]
        vih_all = [sb.tile([H, T, G], fp32, name=f"vih{l}") for l in range(2)]

        zeros_tg = sb.tile([H, T * G], fp32, name="zeros_tg")
        nc.vector.memset(zeros_tg[:], 0.0)
        eps_t = sb.tile([H, 1], fp32, name="eps_t")
        nc.vector.memset(eps_t[:], EPS)

        # ---------------- phase 0: wi0 = x @ w_ih0 (+stats) ----------------
        def phase0():
          for t in range(T):
            xt = loop.tile([I, BL], fp32, tag="xt", bufs=3, name="xt")
            nc.sync.dma_start(xt[:], xT[:, t, :])
            pw = psum.tile([H, G, BL], fp32, tag="gp", name="pw")
            for q in range(G):
                nc.tensor.matmul(pw[:, q, :], wih0_sb[:, q * H:(q + 1) * H],
                                 xt[:], start=True, stop=True)
            for q in range(G):
                nc.vector.bn_stats(st_all[0][:, t, q, :], pw[:, q, :])
            wisb = loop.tile([H, G, BL], fp32, tag="wisb", bufs=3, name="wisb")
            nc.scalar.copy(wisb[:], pw[:])
            nc.sync.dma_start(wi0_dram[t], wisb[:])

        # ---- aggregate wi-layer stats across cores (one AG per layer) ----
        def aggregate_wi_stats(l):
            mv_all = sb.tile([H, T, G, 2], fp32, tag="mv_all", name="mv_all")
            if local_stats:
                for t_ in range(T):
                    for q in range(G):
                        nc.vector.bn_aggr(mv_all[:, t_, q, :],
                                          st_all[l][:, t_, q, :])
            else:
                sbin = dram.tile([H, T * G * 6], fp32, tag=f"wistb{l}",
                                 name="sbin")
                sbout = dram.tile([NCORES * H, T * G * 6], fp32,
                                  tag=f"wistbo{l}", addr_space="Shared",
                                  name="sbout")
                nc.sync.dma_start(
                    sbin[:], st_all[l][:].rearrange("p t g s -> p (t g s)"))
                nc.gpsimd.collective_compute(
                    "AllGather", ALU.bypass, replica_groups=RG,
                    ins=[sbin[:]], outs=[sbout[:]],
                )
                gview = sbout[:].rearrange("(r p) (t s) -> p r t s", r=NCORES,
                                           s=G * 6)
                CH = 19  # 152 = 8*19
                for c0 in range(0, T, CH):
                    gst = loop.tile([H, NCORES, CH, G * 6], fp32, tag="gst",
                                    name="gst")
                    nc.sync.dma_start(gst[:], gview[:, :, c0:c0 + CH, :])
                    for tl in range(CH):
                        for q in range(G):
                            nc.vector.bn_aggr(
                                mv_all[:, c0 + tl, q, :],
                                gst[:, :, tl, 6 * q:6 * (q + 1)])
            # broadcast params along t via activation-bias trick
            def bcast(src):  # src (H, G) -> (H, T, G)
                out = loop.tile([H, T, G], fp32, tag="bc", bufs=4, name="bc")
                for q in range(G):
                    nc.scalar.activation(out[:, :, q], zeros_tg[:, 0:T],
                                         AF.Identity, bias=src[:, q:q + 1])
                return out

            gih_bc = bcast(P[f"gih{l}"])
            ratio_bc = bcast(ratio[l])
            beihb_bc = bcast(P[f"beihb{l}"])
            mean_v = mv_all[:, :, :, 0]
            var_v = mv_all[:, :, :, 1]
            sd = loop.tile([H, T, G], fp32, tag="sd_all", name="sd")
            nc.scalar.activation(sd[:], var_v, AF.Sqrt, bias=eps_t[:])
            r_ = loop.tile([H, T, G], fp32, tag="r_all", name="r_")
            nc.vector.reciprocal(r_[:], sd[:])
            u_ = loop.tile([H, T, G], fp32, tag="u_all", name="u_")
            nc.vector.tensor_tensor(u_[:], r_[:], gih_bc[:], op=ALU.mult)
            nc.vector.tensor_tensor(ugam_all[l][:], r_[:], ratio_bc[:],
                                    op=ALU.mult)
            tmp = loop.tile([H, T, G], fp32, tag="tmp_all", name="tmp")
            nc.vector.tensor_tensor(tmp[:], mean_v, u_[:], op=ALU.mult)
            nc.vector.tensor_tensor(vih_all[l][:], beihb_bc[:], tmp[:],
                                    op=ALU.subtract)

        # ---------------- recurrence ----------------
        def recurrence(l, whh_sb, wi_src_dram, produce_wi1):
            h = loop.tile([H, BL], fp32, tag=f"h{l}", name="h")
            c = loop.tile([H, BL], fp32, tag=f"c{l}", name="c")
            nc.vector.memset(h[:], 0.0)
            nc.vector.memset(c[:], 0.0)
            ghh, behh = P[f"ghh{l}"], P[f"behh{l}"]
            gc_, bec_ = P[f"gc{l}"], P[f"bec{l}"]
            for t in range(T):
                wi_t = loop.tile([H, G, BL], fp32, tag=f"wi_t{l}", bufs=3,
                                 name="wi_t")
                nc.sync.dma_start(wi_t[:], wi_src_dram[t])
                # wh = h @ w_hh  -> PSUM (H, G, BL)
                pw = psum.tile([H, G, BL], fp32, tag="gp", name="pwr")
                for q in range(G):
                    nc.tensor.matmul(pw[:, q, :], whh_sb[:, q * H:(q + 1) * H],
                                     h[:], start=True, stop=True)
                # wh stats -> AG -> aggr
                stw = loop.tile([H, G, 6], fp32, tag="stw", name="stw")
                for q in range(G):
                    nc.vector.bn_stats(stw[:, q, :], pw[:, q, :])
                mv = loop.tile([H, G, 2], fp32, tag="mv", name="mv")
                if local_stats:
                    for q in range(G):
                        nc.vector.bn_aggr(mv[:, q, :], stw[:, q, :])
                else:
                    gbin = dram.tile([H, G * 6], fp32, tag="gbin", name="gbin")
                    gbout = dram.tile([NCORES * H, G * 6], fp32, tag="gbout",
                                      addr_space="Shared", name="gbout")
                    nc.sync.dma_start(gbin[:],
                                      stw[:].rearrange("p g s -> p (g s)"))
                    nc.gpsimd.collective_compute(
                        "AllGather", ALU.bypass, replica_groups=RG,
                        ins=[gbin[:]], outs=[gbout[:]],
                    )
                    gst8 = loop.tile([H, NCORES, G * 6], fp32, tag="gst8",
                                     name="gst8")
                    nc.sync.dma_start(gst8[:],
                                      gbout[:].rearrange("(r p) s -> p r s",
                                                         r=NCORES))
                    for q in range(G):
                        nc.vector.bn_aggr(mv[:, q, :],
                                          gst8[:, :, 6 * q:6 * (q + 1)])
                # s = ghh / sqrt(v+eps); u' = ugam_t * sd; v' = behh - m*s + vih_t
                sd = loop.tile([H, G], fp32, tag="sd", name="sd")
                nc.scalar.activation(sd[:], mv[:, :, 1], AF.Sqrt, bias=eps_t[:])
                s_ = loop.tile([H, G], fp32, tag="s_", name="s_")
                rr = loop.tile([H, G], fp32, tag="rr", name="rr")
                nc.vector.reciprocal(rr[:], sd[:])
                nc.vector.tensor_tensor(s_[:], rr[:], ghh[:], op=ALU.mult)
                up = loop.tile([H, G], fp32, tag="up", name="up")
                nc.vector.tensor_tensor(up[:], ugam_all[l][:, t, :], sd[:],
                                        op=ALU.mult)
                vp = loop.tile([H, G], fp32, tag="vp", name="vp")
                tmg = loop.tile([H, G], fp32, tag="tmg", name="tmg")
                nc.vector.tensor_tensor(tmg[:], mv[:, :, 0], s_[:], op=ALU.mult)
                nc.vector.tensor_tensor(vp[:], behh[:], tmg[:], op=ALU.subtract)
                nc.vector.tensor_tensor(vp[:], vp[:], vih_all[l][:, t, :],
                                        op=ALU.add)
                # gates
                X = loop.tile([H, G, BL], fp32, tag="X", name="X")
                for q in range(G):
                    nc.vector.scalar_tensor_tensor(
                        X[:, q, :], in0=wi_t[:, q, :], scalar=up[:, q:q + 1],
                        in1=pw[:, q, :], op0=ALU.mult, op1=ALU.add)
                ga = loop.tile([H, G, BL], fp32, tag="ga", name="ga")
                for q, fn in enumerate((AF.Sigmoid, AF.Sigmoid, AF.Sigmoid,
                                        AF.Tanh)):
                    nc.scalar.activation(ga[:, q, :], X[:, q, :], fn,
                                         bias=vp[:, q:q + 1],
                                         scale=s_[:, q:q + 1])
                # c1 = f*c + i*g ; h1 = o * tanh(bn(c1))
                t1 = loop.tile([H, BL], fp32, tag="t1", name="t1")
                nc.vector.tensor_tensor(t1[:], ga[:, 1, :], ga[:, 3, :],
                                        op=ALU.mult)
                t2 = loop.tile([H, BL], fp32, tag="t2", name="t2")
                nc.vector.tensor_tensor(t2[:], ga[:, 0, :], c[:], op=ALU.mult)
                c = loop.tile([H, BL], fp32, tag=f"c{l}", name="c")
                nc.vector.tensor_tensor(c[:], t1[:], t2[:], op=ALU.add)
                stc = loop.tile([H, 6], fp32, tag="stc", name="stc")
                nc.vector.bn_stats(stc[:], c[:])
                mvc = loop.tile([H, 2], fp32, tag="mvc", name="mvc")
                if local_stats:
                    nc.vector.bn_aggr(mvc[:], stc[:])
                else:
                    cbin = dram.tile([H, 6], fp32, tag="cbin", name="cbin")
                    cbout = dram.tile([NCORES * H, 6], fp32, tag="cbout",
                                      addr_space="Shared", name="cbout")
                    nc.sync.dma_start(cbin[:], stc[:])
                    nc.gpsimd.collective_compute(
                        "AllGather", ALU.bypass, replica_groups=RG,
                        ins=[cbin[:]], outs=[cbout[:]],
                    )
                    gstc = loop.tile([H, NCORES, 6], fp32, tag="gstc",
                                     name="gstc")
                    nc.sync.dma_start(gstc[:],
                                      cbout[:].rearrange("(r p) s -> p r s",
                                                         r=NCORES))
                    nc.vector.bn_aggr(mvc[:], gstc[:])
                sdc = loop.tile([H, 1], fp32, tag="sdc", name="sdc")
                nc.scalar.activation(sdc[:], mvc[:, 1:2], AF.Sqrt, bias=eps_t[:])
                rc = loop.tile([H, 1], fp32, tag="rc", name="rc")
                nc.vector.reciprocal(rc[:], sdc[:])
                sc = loop.tile([H, 1], fp32, tag="sc", name="sc")
                nc.vector.tensor_tensor(sc[:], rc[:], gc_[:], op=ALU.mult)
                tmc = loop.tile([H, 1], fp32, tag="tmc", name="tmc")
                nc.vector.tensor_tensor(tmc[:], mvc[:, 0:1], sc[:], op=ALU.mult)
                shc = loop.tile([H, 1], fp32, tag="shc", name="shc")
                nc.vector.tensor_tensor(shc[:], bec_[:], tmc[:],
                                        op=ALU.subtract)
                tnc = loop.tile([H, BL], fp32, tag="tnc", name="tnc")
                nc.scalar.activation(tnc[:], c[:], AF.Tanh, bias=shc[:],
                                     scale=sc[:])
                h = loop.tile([H, BL], fp32, tag=f"h{l}", name="h")
                nc.vector.tensor_tensor(h[:], ga[:, 2, :], tnc[:], op=ALU.mult)
                if produce_wi1:
                    pw1 = psum.tile([H, G, BL], fp32, tag="wi1p", name="pw1")
                    for q in range(G):
                        nc.tensor.matmul(pw1[:, q, :],
                                         wih1_sb[:, q * H:(q + 1) * H],
                                         h[:], start=True, stop=True)
                    for q in range(G):
                        nc.vector.bn_stats(st_all[1][:, t, q, :], pw1[:, q, :])
                    wo = loop.tile([H, G, BL], fp32, tag="wo", bufs=3,
                                   name="wo")
                    nc.scalar.copy(wo[:], pw1[:])
                    nc.sync.dma_start(wi1_dram[t], wo[:])
            return h

        for _rep in range(repeats):
            phase0()
            aggregate_wi_stats(0)
            recurrence(0, whh0_sb, wi0_dram, produce_wi1=True)
            aggregate_wi_stats(1)
            h_fin = recurrence(1, whh1_sb, wi1_dram, produce_wi1=False)

        # ---------------- final FC: y = h_fin.T @ fcwT ----------------
        for ci in range(2):
            pf = psum.tile([H, O], fp32, tag="wi1p", name="pf")
            nc.tensor.matmul(pf[:], h_fin[:, ci * H:(ci + 1) * H], fcwT_sb[:],
                             start=True, stop=True)
            yo = loop.tile([H, O], fp32, tag="yo", name="yo")
            nc.scalar.copy(yo[:], pf[:])
            nc.sync.dma_start(
                y[:].rearrange("(c p) o -> c p o", c=2)[ci], yo[:])

    nc.compile()
    return nc


def _build_v2(local_stats=False, repeats=1, skip_phase0=False,
              skip_mainloop=False):
    """Interleaved two-layer pipeline: at step t, layer0 processes timestep t
    while layer1 processes timestep t-1. The per-step BN statistics of
    {wh0[t], wh1[t-1], wi1[t-1]} ride ONE AllGather, and {c0[t], c1[t-1]}
    ride a second — 2 collectives per step for both layers (vs 4)."""
    nc = bacc.Bacc("TRN2", target_bir_lowering=False, debug=False,
                   num_devices=NCORES)

    xT = nc.dram_tensor("xT", [I, T, BL], fp32, kind="ExternalInput").ap()
    wih0 = nc.dram_tensor("wih0", [I, G * H], fp32, kind="ExternalInput").ap()
    whh0 = nc.dram_tensor("whh0", [H, G * H], fp32, kind="ExternalInput").ap()
    wih1 = nc.dram_tensor("wih1", [H, G * H], fp32, kind="ExternalInput").ap()
    whh1 = nc.dram_tensor("whh1", [H, G * H], fp32, kind="ExternalInput").ap()
    fcwT = nc.dram_tensor("fcwT", [H, O], fp32, kind="ExternalInput").ap()
    pin = {}
    for nm, sh in (("gih0", [H, G]), ("beihb0", [H, G]), ("ratio0", [H, G]),
                   ("ratio1", [H, G]), ("gcat12", [H, 12]),
                   ("bcat12", [H, 12]), ("gc2", [H, 2]), ("bc2", [H, 2])):
        pin[nm] = nc.dram_tensor(nm, sh, fp32, kind="ExternalInput").ap()
    y = nc.dram_tensor("y", [BL, O], fp32, kind="ExternalOutput").ap()

    with tile.TileContext(nc) as tc, ExitStack() as ctx:
        sb = ctx.enter_context(tc.tile_pool(name="sb", bufs=1))
        loop = ctx.enter_context(tc.tile_pool(name="loop", bufs=2))
        psum = ctx.enter_context(tc.tile_pool(name="psum", bufs=1, space="PSUM"))
        dram = ctx.enter_context(tc.tile_pool(name="dram", bufs=2, space="DRAM"))

        def load(ap_in, shape, name):
            t_ = sb.tile(shape, fp32, name=name)
            nc.sync.dma_start(t_[:], ap_in[:])
            return t_

        wih0_sb = load(wih0, [I, G * H], "wih0_sb")
        whh0_sb = load(whh0, [H, G * H], "whh0_sb")
        wih1_sb = load(wih1, [H, G * H], "wih1_sb")
        whh1_sb = load(whh1, [H, G * H], "whh1_sb")
        fcwT_sb = load(fcwT, [H, O], "fcwT_sb")
        P = {nm: load(pin[nm], [H, {"gcat12": 12, "bcat12": 12,
                                    "gc2": 2, "bc2": 2}.get(nm, G)], nm + "_sb")
             for nm in pin}

        wi0_dram = dram.tile([T, H, G, BL], fp32, bufs=1, name="wi0_dram")
        st0_all = sb.tile([H, T, G, 6], fp32, name="st0_all")
        ugam0_all = sb.tile([H, T, G], fp32, name="ugam0_all")
        vih0_all = sb.tile([H, T, G], fp32, name="vih0_all")
        zeros_tg = sb.tile([H, T * G], fp32, name="zeros_tg")
        nc.vector.memset(zeros_tg[:], 0.0)
        eps_t = sb.tile([H, 1], fp32, name="eps_t")
        nc.vector.memset(eps_t[:], EPS)

        def phase0():
            for t in range(T):
                xt = loop.tile([I, BL], fp32, tag="xt", bufs=3, name="xt")
                nc.sync.dma_start(xt[:], xT[:, t, :])
                pw = psum.tile([H, G, BL], fp32, tag="g0", bufs=2, name="pw")
                for q in range(G):
                    nc.tensor.matmul(pw[:, q, :], wih0_sb[:, q * H:(q + 1) * H],
                                     xt[:], start=True, stop=True)
                wisb = loop.tile([H, G, BL], fp32, tag="wisb", bufs=3,
                                 name="wisb")
                nc.scalar.copy(wisb[:], pw[:])
                # stats from the SBUF copy: single-src 2x mode, half the cost
                for q in range(G):
                    nc.vector.bn_stats(st0_all[:, t, q, :], wisb[:, q, :])
                nc.sync.dma_start(wi0_dram[t], wisb[:])

        def aggregate0():
            mv_all = sb.tile([H, T, G, 2], fp32, tag="mv_all", name="mv_all")
            if local_stats:
                for t_ in range(T):
                    for q in range(G):
                        nc.vector.bn_aggr(mv_all[:, t_, q, :],
                                          st0_all[:, t_, q, :])
            else:
                sbin = dram.tile([H, T * G * 6], fp32, tag="wistb",
                                 name="sbin")
                sbout = dram.tile([NCORES * H, T * G * 6], fp32, tag="wistbo",
                                  addr_space="Shared", name="sbout")
                nc.sync.dma_start(
                    sbin[:], st0_all[:].rearrange("p t g s -> p (t g s)"))
                nc.gpsimd.collective_compute(
                    "AllGather", ALU.bypass, replica_groups=RG,
                    ins=[sbin[:]], outs=[sbout[:]],
                )
                gview = sbout[:].rearrange("(r p) (t s) -> p r t s", r=NCORES,
                                           s=G * 6)
                CH = 19
                for c0_ in range(0, T, CH):
                    gst = loop.tile([H, NCORES, CH, G * 6], fp32, tag="gst",
                                    name="gst")
                    nc.sync.dma_start(gst[:], gview[:, :, c0_:c0_ + CH, :])
                    for tl in range(CH):
                        for q in range(G):
                            nc.vector.bn_aggr(
                                mv_all[:, c0_ + tl, q, :],
                                gst[:, :, tl, 6 * q:6 * (q + 1)])

            def bcast(src):
                out = loop.tile([H, T, G], fp32, tag="bc", bufs=4, name="bc")
                for q in range(G):
                    nc.scalar.activation(out[:, :, q], zeros_tg[:, 0:T],
                                         AF.Identity, bias=src[:, q:q + 1])
                return out

            gih_bc = bcast(P["gih0"])
            ratio_bc = bcast(P["ratio0"])
            beihb_bc = bcast(P["beihb0"])
            mean_v = mv_all[:, :, :, 0]
            var_v = mv_all[:, :, :, 1]
            sd = loop.tile([H, T, G], fp32, tag="sd_all", name="sd")
            nc.scalar.activation(sd[:], var_v, AF.Sqrt, bias=eps_t[:])
            r_ = loop.tile([H, T, G], fp32, tag="r_all", name="r_")
            nc.vector.reciprocal(r_[:], sd[:])
            u_ = loop.tile([H, T, G], fp32, tag="u_all", name="u_")
            nc.vector.tensor_tensor(u_[:], r_[:], gih_bc[:], op=ALU.mult)
            nc.vector.tensor_tensor(ugam0_all[:], r_[:], ratio_bc[:],
                                    op=ALU.mult)
            tmp = loop.tile([H, T, G], fp32, tag="tmp_all", name="tmp")
            nc.vector.tensor_tensor(tmp[:], mean_v, u_[:], op=ALU.mult)
            nc.vector.tensor_tensor(vih0_all[:], beihb_bc[:], tmp[:],
                                    op=ALU.subtract)

        def mainloop():
            h0 = loop.tile([H, BL], fp32, tag="h0", name="h0")
            c0 = loop.tile([H, BL], fp32, tag="c0", name="c0")
            h1 = loop.tile([H, BL], fp32, tag="h1", name="h1")
            c1 = loop.tile([H, BL], fp32, tag="c1", name="c1")
            for t_ in (h0, c0, h1, c1):
                nc.vector.memset(t_[:], 0.0)
            wi1sb = None
            staga = loop.tile([H, 3, G, 6], fp32, tag="staga", bufs=2,
                              name="staga")
            nc.vector.memset(staga[:], 0.0)
            for t in range(T + 1):
                has0 = t < T
                has1 = t >= 1
                pw0 = pw1 = None
                if has0:
                    wi_t = loop.tile([H, G, BL], fp32, tag="wi0l", bufs=3,
                                     name="wi_t")
                    nc.sync.dma_start(wi_t[:], wi0_dram[t])
                    pw0 = psum.tile([H, G, BL], fp32, tag="g0", bufs=2,
                                    name="pw0")
                    for q in range(G):
                        nc.tensor.matmul(pw0[:, q, :],
                                         whh0_sb[:, q * H:(q + 1) * H],
                                         h0[:], start=True, stop=True)
                    for q in range(G):
                        nc.vector.bn_stats(staga[:, 0, q, :], pw0[:, q, :])
                if has1:
                    pw1 = psum.tile([H, G, BL], fp32, tag="g1", bufs=1,
                                    name="pw1")
                    for q in range(G):
                        nc.tensor.matmul(pw1[:, q, :],
                                         whh1_sb[:, q * H:(q + 1) * H],
                                         h1[:], start=True, stop=True)
                    for q in range(G):
                        nc.vector.bn_stats(staga[:, 1, q, :], pw1[:, q, :])
                # ---- AGa: {wh0, wh1, wi1} stats ----
                mv12 = loop.tile([H, 12, 2], fp32, tag="mv12", name="mv12")
                if local_stats:
                    for k in range(12):
                        nc.vector.bn_aggr(mv12[:, k, :],
                                          staga[:, k // 4, k % 4, :])
                else:
                    abin = dram.tile([H, 72], fp32, tag="abin", name="abin")
                    about = dram.tile([NCORES * H, 72], fp32, tag="about",
                                      addr_space="Shared", name="about")
                    nc.sync.dma_start(abin[:],
                                      staga[:].rearrange("p a g s -> p (a g s)"))
                    nc.gpsimd.collective_compute(
                        "AllGather", ALU.bypass, replica_groups=RG,
                        ins=[abin[:]], outs=[about[:]],
                    )
                    ag8 = loop.tile([H, NCORES, 72], fp32, tag="ag8",
                                    name="ag8")
                    nc.sync.dma_start(ag8[:],
                                      about[:].rearrange("(r p) s -> p r s",
                                                         r=NCORES))
                    for k in range(12):
                        nc.vector.bn_aggr(mv12[:, k, :],
                                          ag8[:, :, 6 * k:6 * (k + 1)])
                sd12 = loop.tile([H, 12], fp32, tag="sd12", name="sd12")
                nc.scalar.activation(sd12[:], mv12[:, :, 1], AF.Sqrt,
                                     bias=eps_t[:])
                r12 = loop.tile([H, 12], fp32, tag="r12", name="r12")
                nc.vector.reciprocal(r12[:], sd12[:])
                S12 = loop.tile([H, 12], fp32, tag="S12", name="S12")
                nc.gpsimd.tensor_tensor(S12[:], r12[:], P["gcat12"][:],
                                        op=ALU.mult)
                TM12 = loop.tile([H, 12], fp32, tag="TM12", name="TM12")
                nc.gpsimd.tensor_tensor(TM12[:], mv12[:, :, 0], S12[:],
                                        op=ALU.mult)
                SH12 = loop.tile([H, 12], fp32, tag="SH12", name="SH12")
                nc.gpsimd.tensor_tensor(SH12[:], P["bcat12"][:], TM12[:],
                                        op=ALU.subtract)
                stagb = loop.tile([H, 2, 6], fp32, tag="stagb", bufs=2,
                                  name="stagb")
                if t == 0:
                    nc.vector.memset(stagb[:, 1, :], 0.0)
                if t == T:
                    nc.vector.memset(stagb[:, 0, :], 0.0)
                ga0 = ga1 = None
                c0n = c1n = None
                if has0:
                    u0p = loop.tile([H, G], fp32, tag="u0p", name="u0p")
                    nc.gpsimd.tensor_tensor(u0p[:], ugam0_all[:, t, :],
                                            sd12[:, 0:4], op=ALU.mult)
                    v0p = loop.tile([H, G], fp32, tag="v0p", name="v0p")
                    nc.gpsimd.tensor_tensor(v0p[:], SH12[:, 0:4],
                                            vih0_all[:, t, :], op=ALU.add)
                    for q in range(G):
                        nc.vector.scalar_tensor_tensor(
                            pw0[:, q, :], in0=wi_t[:, q, :],
                            scalar=u0p[:, q:q + 1], in1=pw0[:, q, :],
                            op0=ALU.mult, op1=ALU.add)
                    ga0 = loop.tile([H, G, BL], fp32, tag="ga0", bufs=2,
                                    name="ga0")
                    for q, fn in enumerate((AF.Sigmoid, AF.Sigmoid,
                                            AF.Sigmoid, AF.Tanh)):
                        nc.scalar.activation(ga0[:, q, :], pw0[:, q, :], fn,
                                             bias=v0p[:, q:q + 1],
                                             scale=S12[:, q:q + 1])
                    t10 = loop.tile([H, BL], fp32, tag="t10", name="t10")
                    nc.vector.tensor_tensor(t10[:], ga0[:, 1, :], ga0[:, 3, :],
                                            op=ALU.mult)
                    t20 = loop.tile([H, BL], fp32, tag="t20", name="t20")
                    nc.gpsimd.tensor_tensor(t20[:], ga0[:, 0, :], c0[:],
                                            op=ALU.mult)
                    c0n = loop.tile([H, BL], fp32, tag="c0", name="c0n")
                    nc.vector.tensor_tensor(c0n[:], t10[:], t20[:], op=ALU.add)
                    nc.vector.bn_stats(stagb[:, 0, :], c0n[:])
                if has1:
                    tu1 = loop.tile([H, G], fp32, tag="tu1", name="tu1")
                    nc.gpsimd.tensor_tensor(tu1[:], P["ratio1"][:],
                                            r12[:, 8:12], op=ALU.mult)
                    u1p = loop.tile([H, G], fp32, tag="u1p", name="u1p")
                    nc.gpsimd.tensor_tensor(u1p[:], tu1[:], sd12[:, 4:8],
                                            op=ALU.mult)
                    v1p = loop.tile([H, G], fp32, tag="v1p", name="v1p")
                    nc.gpsimd.tensor_tensor(v1p[:], SH12[:, 4:8], SH12[:, 8:12],
                                            op=ALU.add)
                    for q in range(G):
                        nc.vector.scalar_tensor_tensor(
                            pw1[:, q, :], in0=wi1sb[:, q, :],
                            scalar=u1p[:, q:q + 1], in1=pw1[:, q, :],
                            op0=ALU.mult, op1=ALU.add)
                    ga1 = loop.tile([H, G, BL], fp32, tag="ga1", bufs=2,
                                    name="ga1")
                    for q, fn in enumerate((AF.Sigmoid, AF.Sigmoid,
                                            AF.Sigmoid, AF.Tanh)):
                        nc.scalar.activation(ga1[:, q, :], pw1[:, q, :], fn,
                                             bias=v1p[:, q:q + 1],
                                             scale=S12[:, 4 + q:5 + q])
                    t11 = loop.tile([H, BL], fp32, tag="t11", name="t11")
                    nc.vector.tensor_tensor(t11[:], ga1[:, 1, :], ga1[:, 3, :],
                                            op=ALU.mult)
                    t21 = loop.tile([H, BL], fp32, tag="t21", name="t21")
                    nc.gpsimd.tensor_tensor(t21[:], ga1[:, 0, :], c1[:],
                                            op=ALU.mult)
                    c1n = loop.tile([H, BL], fp32, tag="c1", name="c1n")
                    nc.vector.tensor_tensor(c1n[:], t11[:], t21[:], op=ALU.add)
                    nc.vector.bn_stats(stagb[:, 1, :], c1n[:])
                # ---- AGb: {c0, c1} stats ----
                mvc2 = loop.tile([H, 2, 2], fp32, tag="mvc2", name="mvc2")
                if local_stats:
                    for k in range(2):
                        nc.vector.bn_aggr(mvc2[:, k, :], stagb[:, k, :])
                else:
                    bbin = dram.tile([H, 12], fp32, tag="bbin", name="bbin")
                    bbout = dram.tile([NCORES * H, 12], fp32, tag="bbout",
                                      addr_space="Shared", name="bbout")
                    nc.sync.dma_start(bbin[:],
                                      stagb[:].rearrange("p a s -> p (a s)"))
                    nc.gpsimd.collective_compute(
                        "AllGather", ALU.bypass, replica_groups=RG,
                        ins=[bbin[:]], outs=[bbout[:]],
                    )
                    bg8 = loop.tile([H, NCORES, 12], fp32, tag="bg8",
                                    name="bg8")
                    nc.sync.dma_start(bg8[:],
                                      bbout[:].rearrange("(r p) s -> p r s",
                                                         r=NCORES))
                    for k in range(2):
                        nc.vector.bn_aggr(mvc2[:, k, :],
                                          bg8[:, :, 6 * k:6 * (k + 1)])
                sdc2 = loop.tile([H, 2], fp32, tag="sdc2", name="sdc2")
                nc.scalar.activation(sdc2[:], mvc2[:, :, 1], AF.Sqrt,
                                     bias=eps_t[:])
                rc2 = loop.tile([H, 2], fp32, tag="rc2", name="rc2")
                nc.vector.reciprocal(rc2[:], sdc2[:])
                scc = loop.tile([H, 2], fp32, tag="scc", name="scc")
                nc.gpsimd.tensor_tensor(scc[:], rc2[:], P["gc2"][:],
                                        op=ALU.mult)
                tmc = loop.tile([H, 2], fp32, tag="tmc", name="tmc")
                nc.gpsimd.tensor_tensor(tmc[:], mvc2[:, :, 0], scc[:],
                                        op=ALU.mult)
                shc = loop.tile([H, 2], fp32, tag="shc", name="shc")
                nc.gpsimd.tensor_tensor(shc[:], P["bc2"][:], tmc[:],
                                        op=ALU.subtract)
                if has0:
                    tn0 = loop.tile([H, BL], fp32, tag="tn0", name="tn0")
                    nc.scalar.activation(tn0[:], c0n[:], AF.Tanh,
                                         bias=shc[:, 0:1], scale=scc[:, 0:1])
                    h0n = loop.tile([H, BL], fp32, tag="h0", name="h0n")
                    nc.gpsimd.tensor_tensor(h0n[:], ga0[:, 2, :], tn0[:],
                                            op=ALU.mult)
                    # produce wi1[t] for layer 1 (consumed next step)
                    pwm = psum.tile([H, G, BL], fp32, tag="w1", bufs=1,
                                    name="pwm")
                    for q in range(G):
                        nc.tensor.matmul(pwm[:, q, :],
                                         wih1_sb[:, q * H:(q + 1) * H],
                                         h0n[:], start=True, stop=True)
                    staga_n = loop.tile([H, 3, G, 6], fp32, tag="staga",
                                        bufs=2, name="staga_n")
                    for q in range(G):
                        nc.vector.bn_stats(staga_n[:, 2, q, :], pwm[:, q, :])
                    wi1n = loop.tile([H, G, BL], fp32, tag="wi1sb", bufs=2,
                                     name="wi1n")
                    nc.scalar.copy(wi1n[:], pwm[:])
                    wi1sb = wi1n
                    staga = staga_n
                    h0 = h0n
                    c0 = c0n
                if has1:
                    tn1 = loop.tile([H, BL], fp32, tag="tn1", name="tn1")
                    nc.scalar.activation(tn1[:], c1n[:], AF.Tanh,
                                         bias=shc[:, 1:2], scale=scc[:, 1:2])
                    h1n = loop.tile([H, BL], fp32, tag="h1", name="h1n")
                    nc.gpsimd.tensor_tensor(h1n[:], ga1[:, 2, :], tn1[:],
                                            op=ALU.mult)
                    h1 = h1n
                    c1 = c1n
            return h1

        h_fin = None
        for _rep in range(repeats):
            if not skip_phase0:
                phase0()
                aggregate0()
            if not skip_mainloop:
                h_fin = mainloop()
        if h_fin is None:
            h_fin = loop.tile([H, BL], fp32, tag="h1", name="hf")
            nc.vector.memset(h_fin[:], 0.0)

        for ci in range(2):
            pf = psum.tile([H, O], fp32, tag="w1", name="pf")
            nc.tensor.matmul(pf[:], h_fin[:, ci * H:(ci + 1) * H], fcwT_sb[:],
                             start=True, stop=True)
            yo = loop.tile([H, O], fp32, tag="yo", name="yo")
            nc.scalar.copy(yo[:], pf[:])
            nc.sync.dma_start(
                y[:].rearrange("(c p) o -> c p o", c=2)[ci], yo[:])

    nc.compile()
    return nc


VERSION = 2

_NC_CACHE = None


def _get_nc():
    global _NC_CACHE
    if _NC_CACHE is None:
        _NC_CACHE = _build_v2() if VERSION == 2 else _build()
    return _NC_CACHE


def _prep_inputs(sequences, w_ih0, w_hh0, b0, g_ih0, be_ih0, g_hh0, be_hh0,
                 g_c0, be_c0, w_ih1, w_hh1, b1, g_ih1, be_ih1, g_hh1, be_hh1,
                 g_c1, be_c1, fc_w, fc_b):
    f32 = np.float32

    def pg(v):  # (512,) -> (128, 4)
        return np.ascontiguousarray(np.asarray(v, f32).reshape(G, H).T)

    common = {
        "wih0": np.ascontiguousarray(np.asarray(w_ih0, f32)),
        "whh0": np.ascontiguousarray(np.asarray(w_hh0, f32)),
        "wih1": np.ascontiguousarray(np.asarray(w_ih1, f32)),
        "whh1": np.ascontiguousarray(np.asarray(w_hh1, f32)),
        "fcwT": np.ascontiguousarray(np.asarray(fc_w, f32).T),
    }
    if VERSION == 2:
        common.update({
            "gih0": pg(g_ih0),
            "beihb0": pg(np.asarray(be_ih0) + np.asarray(b0)),
            "ratio0": pg(np.asarray(g_ih0) / np.asarray(g_hh0)),
            "ratio1": pg(np.asarray(g_ih1) / np.asarray(g_hh1)),
            "gcat12": np.concatenate([pg(g_hh0), pg(g_hh1), pg(g_ih1)],
                                     axis=1),
            "bcat12": np.concatenate(
                [pg(be_hh0), pg(be_hh1),
                 pg(np.asarray(be_ih1) + np.asarray(b1))], axis=1),
            "gc2": np.stack([np.asarray(g_c0, f32),
                             np.asarray(g_c1, f32)], axis=1).copy(),
            "bc2": np.stack([np.asarray(be_c0, f32),
                             np.asarray(be_c1, f32)], axis=1).copy(),
        })
    else:
        common.update({
            "gih0": pg(g_ih0),
            "beihb0": pg(np.asarray(be_ih0) + np.asarray(b0)),
            "ghh0": pg(g_hh0), "behh0": pg(be_hh0),
            "gc0": np.asarray(g_c0, f32).reshape(H, 1).copy(),
            "bec0": np.asarray(be_c0, f32).reshape(H, 1).copy(),
            "gih1": pg(g_ih1),
            "beihb1": pg(np.asarray(be_ih1) + np.asarray(b1)),
            "ghh1": pg(g_hh1), "behh1": pg(be_hh1),
            "gc1": np.asarray(g_c1, f32).reshape(H, 1).copy(),
            "bec1": np.asarray(be_c1, f32).reshape(H, 1).copy(),
        })
    seq = np.asarray(sequences, f32)
    in_maps = []
    for c in range(NCORES):
        m = dict(common)
        m["xT"] = np.ascontiguousarray(
            seq[c * BL:(c + 1) * BL].transpose(2, 1, 0))  # (I, T, BL)
        in_maps.append(m)
    return in_maps


def kernel(**inputs):
    nc = _get_nc()
    in_maps = _prep_inputs(**inputs)
    last_exc = None
    for attempt in range(3):
        try:
            res = run_bass_kernel_spmd(nc, in_maps,
                                       core_ids=list(range(NCORES)),
                                       trace=False)
            break
        except Exception as e:  # transient runtime INTERNAL errors observed
            last_exc = e
            time.sleep(2.0)
    else:
        raise last_exc
    ys = [res.results[c]["y"] for c in range(NCORES)]
    out = np.concatenate(ys, axis=0)  # (B, O)
    out = out + np.asarray(inputs["fc_b"], np.float32)[None, :]
    return out.astype(np.float32)


# revision 28
# speedup vs baseline: 10.4296x; 9.1708x over previous
"""BN-LSTM (2-layer, Cooijmans) Trainium2 Bass kernel, 8-way batch-parallel.

Problem: B=2048, T=152, I=75, H=128, O=256. Training-mode BatchNorm over the
batch axis inside the recurrence => per-timestep cross-core statistics.

Strategy:
- Data-parallel over batch: 256 rows/core, params replicated.
- Layout: features on partitions, local batch on the free dim.
  h/c state tiles are (128=H, 256=B_loc); gate pre-acts are (128, 4, 256).
- Exact BN parity: per-step partial stats via vector.bn_stats, AllGathered
  across the 8 cores (DRAM bounce), combined with vector.bn_aggr.
- Input projections wi = x @ w_ih are computed in a bulk phase (layer-0 stats
  shipped in ONE AllGather); their BN is folded into the recurrence as
  pre_q = s_q*(wh_q + u'_q*wi_q) + v'_q with
    s_q   = gamma_hh/sd_hh          (per-step, from wh stats)
    u'_q  = (gamma_ih/gamma_hh) * sd_hh / sd_ih   (per-step scalar per feature)
    v'_q  = (beta_hh - m_hh*s) + (beta_ih + b - m_ih*u_ih)
  so each gate costs one vector scalar_tensor_tensor (in-place into PSUM) +
  one scalar activation with per-partition scale/bias.
- v2 (used): the two layers run interleaved (layer1 lags one timestep), so
  ONE AllGather carries {wh0[t], wh1[t-1], wi1[t-1]} stats and a second
  carries {c0[t], c1[t-1]} — 2 collectives/step for both layers instead of 4,
  and layer1's wi projections stay in SBUF (no DRAM round trip). Elementwise
  work is split across Vector/GpSimd/Scalar engines.
Measured on the 8-core axon TRN2 environment: ~8.4 ms end-to-end (slope
method; wall-clock per dispatch carries ~80ms of fixed PJRT overhead),
relative error vs the fp32 reference ~1.6e-6.
"""

import time
from contextlib import ExitStack

import numpy as np

import concourse.bass as bass
import concourse.mybir as mybir
import concourse.bacc as bacc
import concourse.tile as tile
from concourse.bass_utils import run_bass_kernel_spmd

# ---- problem constants (hardcoded per harness contract) ----
B, T, I, H, O = 2048, 152, 75, 128, 256
NCORES = 8
BL = B // NCORES  # 256 local batch
G = 4             # gates f, i, o, g
EPS = 1e-5

fp32 = mybir.dt.float32
AF = mybir.ActivationFunctionType
ALU = mybir.AluOpType
RG = [list(range(NCORES))]


def _build(local_stats=False, repeats=1):
    """local_stats=True: numerically WRONG (per-shard BN) — timing probe only.
    repeats>1: run the whole pipeline N times serially (timing slope probe)."""
    nc = bacc.Bacc("TRN2", target_bir_lowering=False, debug=False,
                   num_devices=NCORES)

    # ---- kernel I/O ----
    xT = nc.dram_tensor("xT", [I, T, BL], fp32, kind="ExternalInput").ap()
    wih0 = nc.dram_tensor("wih0", [I, G * H], fp32, kind="ExternalInput").ap()
    whh0 = nc.dram_tensor("whh0", [H, G * H], fp32, kind="ExternalInput").ap()
    wih1 = nc.dram_tensor("wih1", [H, G * H], fp32, kind="ExternalInput").ap()
    whh1 = nc.dram_tensor("whh1", [H, G * H], fp32, kind="ExternalInput").ap()
    fcwT = nc.dram_tensor("fcwT", [H, O], fp32, kind="ExternalInput").ap()
    pin = {}
    for l in range(2):
        for nm, sh in (("gih", [H, G]), ("beihb", [H, G]), ("ghh", [H, G]),
                       ("behh", [H, G]), ("gc", [H, 1]), ("bec", [H, 1])):
            key = f"{nm}{l}"
            pin[key] = nc.dram_tensor(key, sh, fp32, kind="ExternalInput").ap()
    y = nc.dram_tensor("y", [BL, O], fp32, kind="ExternalOutput").ap()

    with tile.TileContext(nc) as tc, ExitStack() as ctx:
        sb = ctx.enter_context(tc.tile_pool(name="sb", bufs=1))
        loop = ctx.enter_context(tc.tile_pool(name="loop", bufs=2))
        psum = ctx.enter_context(tc.tile_pool(name="psum", bufs=2, space="PSUM"))
        dram = ctx.enter_context(tc.tile_pool(name="dram", bufs=2, space="DRAM"))

        # ---- load params to SBUF ----
        def load(ap_in, shape, name):
            t_ = sb.tile(shape, fp32, name=name)
            nc.sync.dma_start(t_[:], ap_in[:])
            return t_

        wih0_sb = load(wih0, [I, G * H], "wih0_sb")
        whh0_sb = load(whh0, [H, G * H], "whh0_sb")
        wih1_sb = load(wih1, [H, G * H], "wih1_sb")
        whh1_sb = load(whh1, [H, G * H], "whh1_sb")
        fcwT_sb = load(fcwT, [H, O], "fcwT_sb")
        P = {}
        for l in range(2):
            for nm in ("gih", "beihb", "ghh", "behh"):
                key = f"{nm}{l}"
                P[key] = load(pin[key], [H, G], key + "_sb")
            for nm in ("gc", "bec"):
                key = f"{nm}{l}"
                P[key] = load(pin[key], [H, 1], key + "_sb")

        # per-layer gamma_ih/gamma_hh ratio
        ratio = {}
        for l in range(2):
            ig = sb.tile([H, G], fp32, name=f"invghh{l}")
            nc.vector.reciprocal(ig[:], P[f"ghh{l}"][:])
            r_ = sb.tile([H, G], fp32, name=f"ratio{l}")
            nc.vector.tensor_tensor(r_[:], P[f"gih{l}"][:], ig[:], op=ALU.mult)
            ratio[l] = r_

        # wi scratch in DRAM (per-core, internal)
        wi0_dram = dram.tile([T, H, G, BL], fp32, bufs=1, name="wi0_dram")
        wi1_dram = dram.tile([T, H, G, BL], fp32, bufs=1, name="wi1_dram")

        # per-(t,gate) wi-BN stats records, for the one-shot AllGather
        st_all = [sb.tile([H, T, G, 6], fp32, name=f"st{l}_all") for l in range(2)]
        # aggregated per-(t,gate) wi mean/var -> folded scales
        ugam_all = [sb.tile([H, T, G], fp32, name=f"ugam{l}") for l in range(2)]
        vih_all = [sb.tile([H, T, G], fp32, name=f"vih{l}") for l in range(2)]

        zeros_tg = sb.tile([H, T * G], fp32, name="zeros_tg")
        nc.vector.memset(zeros_tg[:], 0.0)
        eps_t = sb.tile([H, 1], fp32, name="eps_t")
        nc.vector.memset(eps_t[:], EPS)

        # ---------------- phase 0: wi0 = x @ w_ih0 (+stats) ----------------
        def phase0():
          for t in range(T):
            xt = loop.tile([I, BL], fp32, tag="xt", bufs=3, name="xt")
            nc.sync.dma_start(xt[:], xT[:, t, :])
            pw = psum.tile([H, G, BL], fp32, tag="gp", name="pw")
            for q in range(G):
                nc.tensor.matmul(pw[:, q, :], wih0_sb[:, q * H:(q + 1) * H],
                                 xt[:], start=True, stop=True)
            for q in range(G):
                nc.vector.bn_stats(st_all[0][:, t, q, :], pw[:, q, :])
            wisb = loop.tile([H, G, BL], fp32, tag="wisb", bufs=3, name="wisb")
            nc.scalar.copy(wisb[:], pw[:])
            nc.sync.dma_start(wi0_dram[t], wisb[:])

        # ---- aggregate wi-layer stats across cores (one AG per layer) ----
        def aggregate_wi_stats(l):
            mv_all = sb.tile([H, T, G, 2], fp32, tag="mv_all", name="mv_all")
            if local_stats:
                for t_ in range(T):
                    for q in range(G):
                        nc.vector.bn_aggr(mv_all[:, t_, q, :],
                                          st_all[l][:, t_, q, :])
            else:
                sbin = dram.tile([H, T * G * 6], fp32, tag=f"wistb{l}",
                                 name="sbin")
                sbout = dram.tile([NCORES * H, T * G * 6], fp32,
                                  tag=f"wistbo{l}", addr_space="Shared",
                                  name="sbout")
                nc.sync.dma_start(
                    sbin[:], st_all[l][:].rearrange("p t g s -> p (t g s)"))
                nc.gpsimd.collective_compute(
                    "AllGather", ALU.bypass, replica_groups=RG,
                    ins=[sbin[:]], outs=[sbout[:]],
                )
                gview = sbout[:].rearrange("(r p) (t s) -> p r t s", r=NCORES,
                                           s=G * 6)
                CH = 19  # 152 = 8*19
                for c0 in range(0, T, CH):
                    gst = loop.tile([H, NCORES, CH, G * 6], fp32, tag="gst",
                                    name="gst")
                    nc.sync.dma_start(gst[:], gview[:, :, c0:c0 + CH, :])
                    for tl in range(CH):
                        for q in range(G):
                            nc.vector.bn_aggr(
                                mv_all[:, c0 + tl, q, :],
                                gst[:, :, tl, 6 * q:6 * (q + 1)])
            # broadcast params along t via activation-bias trick
            def bcast(src):  # src (H, G) -> (H, T, G)
                out = loop.tile([H, T, G], fp32, tag="bc", bufs=4, name="bc")
                for q in range(G):
                    nc.scalar.activation(out[:, :, q], zeros_tg[:, 0:T],
                                         AF.Identity, bias=src[:, q:q + 1])
                return out

            gih_bc = bcast(P[f"gih{l}"])
            ratio_bc = bcast(ratio[l])
            beihb_bc = bcast(P[f"beihb{l}"])
            mean_v = mv_all[:, :, :, 0]
            var_v = mv_all[:, :, :, 1]
            sd = loop.tile([H, T, G], fp32, tag="sd_all", name="sd")
            nc.scalar.activation(sd[:], var_v, AF.Sqrt, bias=eps_t[:])
            r_ = loop.tile([H, T, G], fp32, tag="r_all", name="r_")
            nc.vector.reciprocal(r_[:], sd[:])
            u_ = loop.tile([H, T, G], fp32, tag="u_all", name="u_")
            nc.vector.tensor_tensor(u_[:], r_[:], gih_bc[:], op=ALU.mult)
            nc.vector.tensor_tensor(ugam_all[l][:], r_[:], ratio_bc[:],
                                    op=ALU.mult)
            tmp = loop.tile([H, T, G], fp32, tag="tmp_all", name="tmp")
            nc.vector.tensor_tensor(tmp[:], mean_v, u_[:], op=ALU.mult)
            nc.vector.tensor_tensor(vih_all[l][:], beihb_bc[:], tmp[:],
                                    op=ALU.subtract)

        # ---------------- recurrence ----------------
        def recurrence(l, whh_sb, wi_src_dram, produce_wi1):
            h = loop.tile([H, BL], fp32, tag=f"h{l}", name="h")
            c = loop.tile([H, BL], fp32, tag=f"c{l}", name="c")
            nc.vector.memset(h[:], 0.0)
            nc.vector.memset(c[:], 0.0)
            ghh, behh = P[f"ghh{l}"], P[f"behh{l}"]
            gc_, bec_ = P[f"gc{l}"], P[f"bec{l}"]
            for t in range(T):
                wi_t = loop.tile([H, G, BL], fp32, tag=f"wi_t{l}", bufs=3,
                                 name="wi_t")
                nc.sync.dma_start(wi_t[:], wi_src_dram[t])
                # wh = h @ w_hh  -> PSUM (H, G, BL)
                pw = psum.tile([H, G, BL], fp32, tag="gp", name="pwr")
                for q in range(G):
                    nc.tensor.matmul(pw[:, q, :], whh_sb[:, q * H:(q + 1) * H],
                                     h[:], start=True, stop=True)
                # wh stats -> AG -> aggr
                stw = loop.tile([H, G, 6], fp32, tag="stw", name="stw")
                for q in range(G):
                    nc.vector.bn_stats(stw[:, q, :], pw[:, q, :])
                mv = loop.tile([H, G, 2], fp32, tag="mv", name="mv")
                if local_stats:
                    for q in range(G):
                        nc.vector.bn_aggr(mv[:, q, :], stw[:, q, :])
                else:
                    gbin = dram.tile([H, G * 6], fp32, tag="gbin", name="gbin")
                    gbout = dram.tile([NCORES * H, G * 6], fp32, tag="gbout",
                                      addr_space="Shared", name="gbout")
                    nc.sync.dma_start(gbin[:],
                                      stw[:].rearrange("p g s -> p (g s)"))
                    nc.gpsimd.collective_compute(
                        "AllGather", ALU.bypass, replica_groups=RG,
                        ins=[gbin[:]], outs=[gbout[:]],
                    )
                    gst8 = loop.tile([H, NCORES, G * 6], fp32, tag="gst8",
                                     name="gst8")
                    nc.sync.dma_start(gst8[:],
                                      gbout[:].rearrange("(r p) s -> p r s",
                                                         r=NCORES))
                    for q in range(G):
                        nc.vector.bn_aggr(mv[:, q, :],
                                          gst8[:, :, 6 * q:6 * (q + 1)])
                # s = ghh / sqrt(v+eps); u' = ugam_t * sd; v' = behh - m*s + vih_t
                sd = loop.tile([H, G], fp32, tag="sd", name="sd")
                nc.scalar.activation(sd[:], mv[:, :, 1], AF.Sqrt, bias=eps_t[:])
                s_ = loop.tile([H, G], fp32, tag="s_", name="s_")
                rr = loop.tile([H, G], fp32, tag="rr", name="rr")
                nc.vector.reciprocal(rr[:], sd[:])
                nc.vector.tensor_tensor(s_[:], rr[:], ghh[:], op=ALU.mult)
                up = loop.tile([H, G], fp32, tag="up", name="up")
                nc.vector.tensor_tensor(up[:], ugam_all[l][:, t, :], sd[:],
                                        op=ALU.mult)
                vp = loop.tile([H, G], fp32, tag="vp", name="vp")
                tmg = loop.tile([H, G], fp32, tag="tmg", name="tmg")
                nc.vector.tensor_tensor(tmg[:], mv[:, :, 0], s_[:], op=ALU.mult)
                nc.vector.tensor_tensor(vp[:], behh[:], tmg[:], op=ALU.subtract)
                nc.vector.tensor_tensor(vp[:], vp[:], vih_all[l][:, t, :],
                                        op=ALU.add)
                # gates
                X = loop.tile([H, G, BL], fp32, tag="X", name="X")
                for q in range(G):
                    nc.vector.scalar_tensor_tensor(
                        X[:, q, :], in0=wi_t[:, q, :], scalar=up[:, q:q + 1],
                        in1=pw[:, q, :], op0=ALU.mult, op1=ALU.add)
                ga = loop.tile([H, G, BL], fp32, tag="ga", name="ga")
                for q, fn in enumerate((AF.Sigmoid, AF.Sigmoid, AF.Sigmoid,
                                        AF.Tanh)):
                    nc.scalar.activation(ga[:, q, :], X[:, q, :], fn,
                                         bias=vp[:, q:q + 1],
                                         scale=s_[:, q:q + 1])
                # c1 = f*c + i*g ; h1 = o * tanh(bn(c1))
                t1 = loop.tile([H, BL], fp32, tag="t1", name="t1")
                nc.vector.tensor_tensor(t1[:], ga[:, 1, :], ga[:, 3, :],
                                        op=ALU.mult)
                t2 = loop.tile([H, BL], fp32, tag="t2", name="t2")
                nc.vector.tensor_tensor(t2[:], ga[:, 0, :], c[:], op=ALU.mult)
                c = loop.tile([H, BL], fp32, tag=f"c{l}", name="c")
                nc.vector.tensor_tensor(c[:], t1[:], t2[:], op=ALU.add)
                stc = loop.tile([H, 6], fp32, tag="stc", name="stc")
                nc.vector.bn_stats(stc[:], c[:])
                mvc = loop.tile([H, 2], fp32, tag="mvc", name="mvc")
                if local_stats:
                    nc.vector.bn_aggr(mvc[:], stc[:])
                else:
                    cbin = dram.tile([H, 6], fp32, tag="cbin", name="cbin")
                    cbout = dram.tile([NCORES * H, 6], fp32, tag="cbout",
                                      addr_space="Shared", name="cbout")
                    nc.sync.dma_start(cbin[:], stc[:])
                    nc.gpsimd.collective_compute(
                        "AllGather", ALU.bypass, replica_groups=RG,
                        ins=[cbin[:]], outs=[cbout[:]],
                    )
                    gstc = loop.tile([H, NCORES, 6], fp32, tag="gstc",
                                     name="gstc")
                    nc.sync.dma_start(gstc[:],
                                      cbout[:].rearrange("(r p) s -> p r s",
                                                         r=NCORES))
                    nc.vector.bn_aggr(mvc[:], gstc[:])
                sdc = loop.tile([H, 1], fp32, tag="sdc", name="sdc")
                nc.scalar.activation(sdc[:], mvc[:, 1:2], AF.Sqrt, bias=eps_t[:])
                rc = loop.tile([H, 1], fp32, tag="rc", name="rc")
                nc.vector.reciprocal(rc[:], sdc[:])
                sc = loop.tile([H, 1], fp32, tag="sc", name="sc")
                nc.vector.tensor_tensor(sc[:], rc[:], gc_[:], op=ALU.mult)
                tmc = loop.tile([H, 1], fp32, tag="tmc", name="tmc")
                nc.vector.tensor_tensor(tmc[:], mvc[:, 0:1], sc[:], op=ALU.mult)
                shc = loop.tile([H, 1], fp32, tag="shc", name="shc")
                nc.vector.tensor_tensor(shc[:], bec_[:], tmc[:],
                                        op=ALU.subtract)
                tnc = loop.tile([H, BL], fp32, tag="tnc", name="tnc")
                nc.scalar.activation(tnc[:], c[:], AF.Tanh, bias=shc[:],
                                     scale=sc[:])
                h = loop.tile([H, BL], fp32, tag=f"h{l}", name="h")
                nc.vector.tensor_tensor(h[:], ga[:, 2, :], tnc[:], op=ALU.mult)
                if produce_wi1:
                    pw1 = psum.tile([H, G, BL], fp32, tag="wi1p", name="pw1")
                    for q in range(G):
                        nc.tensor.matmul(pw1[:, q, :],
                                         wih1_sb[:, q * H:(q + 1) * H],
                                         h[:], start=True, stop=True)
                    for q in range(G):
                        nc.vector.bn_stats(st_all[1][:, t, q, :], pw1[:, q, :])
                    wo = loop.tile([H, G, BL], fp32, tag="wo", bufs=3,
                                   name="wo")
                    nc.scalar.copy(wo[:], pw1[:])
                    nc.sync.dma_start(wi1_dram[t], wo[:])
            return h

        for _rep in range(repeats):
            phase0()
            aggregate_wi_stats(0)
            recurrence(0, whh0_sb, wi0_dram, produce_wi1=True)
            aggregate_wi_stats(1)
            h_fin = recurrence(1, whh1_sb, wi1_dram, produce_wi1=False)

        # ---------------- final FC: y = h_fin.T @ fcwT ----------------
        for ci in range(2):
            pf = psum.tile([H, O], fp32, tag="wi1p", name="pf")
            nc.tensor.matmul(pf[:], h_fin[:, ci * H:(ci + 1) * H], fcwT_sb[:],
                             start=True, stop=True)
            yo = loop.tile([H, O], fp32, tag="yo", name="yo")
            nc.scalar.copy(yo[:], pf[:])
            nc.sync.dma_start(
                y[:].rearrange("(c p) o -> c p o", c=2)[ci], yo[:])

    nc.compile()
    return nc


def _build_v2(local_stats=False, repeats=1, skip_phase0=False,
              skip_mainloop=False):
    """Interleaved two-layer pipeline: at step t, layer0 processes timestep t
    while layer1 processes timestep t-1. The per-step BN statistics of
    {wh0[t], wh1[t-1], wi1[t-1]} ride ONE AllGather, and {c0[t], c1[t-1]}
    ride a second — 2 collectives per step for both layers (vs 4)."""
    nc = bacc.Bacc("TRN2", target_bir_lowering=False, debug=False,
                   num_devices=NCORES)

    xT = nc.dram_tensor("xT", [I, T, BL], fp32, kind="ExternalInput").ap()
    wih0 = nc.dram_tensor("wih0", [I, G * H], fp32, kind="ExternalInput").ap()
    whh0 = nc.dram_tensor("whh0", [H, G * H], fp32, kind="ExternalInput").ap()
    wih1 = nc.dram_tensor("wih1", [H, G * H], fp32, kind="ExternalInput").ap()
    whh1 = nc.dram_tensor("whh1", [H, G * H], fp32, kind="ExternalInput").ap()
    fcwT = nc.dram_tensor("fcwT", [H, O], fp32, kind="ExternalInput").ap()
    pin = {}
    for nm, sh in (("gih0", [H, G]), ("beihb0", [H, G]), ("ratio0", [H, G]),
                   ("ratio1", [H, G]), ("gcat12", [H, 12]),
                   ("bcat12", [H, 12]), ("gc2", [H, 2]), ("bc2", [H, 2])):
        pin[nm] = nc.dram_tensor(nm, sh, fp32, kind="ExternalInput").ap()
    y = nc.dram_tensor("y", [BL, O], fp32, kind="ExternalOutput").ap()

    with tile.TileContext(nc) as tc, ExitStack() as ctx:
        sb = ctx.enter_context(tc.tile_pool(name="sb", bufs=1))
        loop = ctx.enter_context(tc.tile_pool(name="loop", bufs=2))
        psum = ctx.enter_context(tc.tile_pool(name="psum", bufs=1, space="PSUM"))
        dram = ctx.enter_context(tc.tile_pool(name="dram", bufs=2, space="DRAM"))

        def load(ap_in, shape, name):
            t_ = sb.tile(shape, fp32, name=name)
            nc.sync.dma_start(t_[:], ap_in[:])
            return t_

        wih0_sb = load(wih0, [I, G * H], "wih0_sb")
        whh0_sb = load(whh0, [H, G * H], "whh0_sb")
        wih1_sb = load(wih1, [H, G * H], "wih1_sb")
        whh1_sb = load(whh1, [H, G * H], "whh1_sb")
        fcwT_sb = load(fcwT, [H, O], "fcwT_sb")
        P = {nm: load(pin[nm], [H, {"gcat12": 12, "bcat12": 12,
                                    "gc2": 2, "bc2": 2}.get(nm, G)], nm + "_sb")
             for nm in pin}

        wi0_dram = dram.tile([T, H, G, BL], fp32, bufs=1, name="wi0_dram")
        st0_all = sb.tile([H, T, G, 6], fp32, name="st0_all")
        ugam0_all = sb.tile([H, T, G], fp32, name="ugam0_all")
        vih0_all = sb.tile([H, T, G], fp32, name="vih0_all")
        zeros_tg = sb.tile([H, T * G], fp32, name="zeros_tg")
        nc.vector.memset(zeros_tg[:], 0.0)
        eps_t = sb.tile([H, 1], fp32, name="eps_t")
        nc.vector.memset(eps_t[:], EPS)

        def phase0():
            for t in range(T):
                xt = loop.tile([I, BL], fp32, tag="xt", bufs=3, name="xt")
                nc.sync.dma_start(xt[:], xT[:, t, :])
                pw = psum.tile([H, G, BL], fp32, tag="g0", bufs=2, name="pw")
                for q in range(G):
                    nc.tensor.matmul(pw[:, q, :], wih0_sb[:, q * H:(q + 1) * H],
                                     xt[:], start=True, stop=True)
                wisb = loop.tile([H, G, BL], fp32, tag="wisb", bufs=3,
                                 name="wisb")
                nc.scalar.copy(wisb[:], pw[:])
                # stats from the SBUF copy: single-src 2x mode, half the cost
                for q in range(G):
                    nc.vector.bn_stats(st0_all[:, t, q, :], wisb[:, q, :])
                nc.sync.dma_start(wi0_dram[t], wisb[:])

        def aggregate0():
            mv_all = sb.tile([H, T, G, 2], fp32, tag="mv_all", name="mv_all")
            if local_stats:
                for t_ in range(T):
                    for q in range(G):
                        nc.vector.bn_aggr(mv_all[:, t_, q, :],
                                          st0_all[:, t_, q, :])
            else:
                sbin = dram.tile([H, T * G * 6], fp32, tag="wistb",
                                 name="sbin")
                sbout = dram.tile([NCORES * H, T * G * 6], fp32, tag="wistbo",
                                  addr_space="Shared", name="sbout")
                nc.sync.dma_start(
                    sbin[:], st0_all[:].rearrange("p t g s -> p (t g s)"))
                nc.gpsimd.collective_compute(
                    "AllGather", ALU.bypass, replica_groups=RG,
                    ins=[sbin[:]], outs=[sbout[:]],
                )
                gview = sbout[:].rearrange("(r p) (t s) -> p r t s", r=NCORES,
                                           s=G * 6)
                CH = 19
                for c0_ in range(0, T, CH):
                    gst = loop.tile([H, NCORES, CH, G * 6], fp32, tag="gst",
                                    name="gst")
                    nc.sync.dma_start(gst[:], gview[:, :, c0_:c0_ + CH, :])
                    for tl in range(CH):
                        for q in range(G):
                            nc.vector.bn_aggr(
                                mv_all[:, c0_ + tl, q, :],
                                gst[:, :, tl, 6 * q:6 * (q + 1)])

            def bcast(src):
                out = loop.tile([H, T, G], fp32, tag="bc", bufs=4, name="bc")
                for q in range(G):
                    nc.scalar.activation(out[:, :, q], zeros_tg[:, 0:T],
                                         AF.Identity, bias=src[:, q:q + 1])
                return out

            gih_bc = bcast(P["gih0"])
            ratio_bc = bcast(P["ratio0"])
            beihb_bc = bcast(P["beihb0"])
            mean_v = mv_all[:, :, :, 0]
            var_v = mv_all[:, :, :, 1]
            sd = loop.tile([H, T, G], fp32, tag="sd_all", name="sd")
            nc.scalar.activation(sd[:], var_v, AF.Sqrt, bias=eps_t[:])
            r_ = loop.tile([H, T, G], fp32, tag="r_all", name="r_")
            nc.vector.reciprocal(r_[:], sd[:])
            u_ = loop.tile([H, T, G], fp32, tag="u_all", name="u_")
            nc.vector.tensor_tensor(u_[:], r_[:], gih_bc[:], op=ALU.mult)
            nc.vector.tensor_tensor(ugam0_all[:], r_[:], ratio_bc[:],
                                    op=ALU.mult)
            tmp = loop.tile([H, T, G], fp32, tag="tmp_all", name="tmp")
            nc.vector.tensor_tensor(tmp[:], mean_v, u_[:], op=ALU.mult)
            nc.vector.tensor_tensor(vih0_all[:], beihb_bc[:], tmp[:],
                                    op=ALU.subtract)

        def mainloop():
            h0 = loop.tile([H, BL], fp32, tag="h0", name="h0")
            c0 = loop.tile([H, BL], fp32, tag="c0", name="c0")
            h1 = loop.tile([H, BL], fp32, tag="h1", name="h1")
            c1 = loop.tile([H, BL], fp32, tag="c1", name="c1")
            for t_ in (h0, c0, h1, c1):
                nc.vector.memset(t_[:], 0.0)
            wi1sb = None
            staga = loop.tile([H, 3, G, 6], fp32, tag="staga", bufs=2,
                              name="staga")
            nc.vector.memset(staga[:], 0.0)
            for t in range(T + 1):
                has0 = t < T
                has1 = t >= 1
                pw0 = pw1 = None
                if has0:
                    wi_t = loop.tile([H, G, BL], fp32, tag="wi0l", bufs=3,
                                     name="wi_t")
                    nc.sync.dma_start(wi_t[:], wi0_dram[t])
                    pw0 = psum.tile([H, G, BL], fp32, tag="g0", bufs=2,
                                    name="pw0")
                    for q in range(G):
                        nc.tensor.matmul(pw0[:, q, :],
                                         whh0_sb[:, q * H:(q + 1) * H],
                                         h0[:], start=True, stop=True)
                    for q in range(G):
                        nc.vector.bn_stats(staga[:, 0, q, :], pw0[:, q, :])
                if has1:
                    pw1 = psum.tile([H, G, BL], fp32, tag="g1", bufs=1,
                                    name="pw1")
                    for q in range(G):
                        nc.tensor.matmul(pw1[:, q, :],
                                         whh1_sb[:, q * H:(q + 1) * H],
                                         h1[:], start=True, stop=True)
                    for q in range(G):
                        nc.vector.bn_stats(staga[:, 1, q, :], pw1[:, q, :])
                # ---- AGa: {wh0, wh1, wi1} stats ----
                mv12 = loop.tile([H, 12, 2], fp32, tag="mv12", name="mv12")
                if local_stats:
                    for k in range(12):
                        nc.vector.bn_aggr(mv12[:, k, :],
                                          staga[:, k // 4, k % 4, :])
                else:
                    abin = dram.tile([H, 72], fp32, tag="abin", name="abin")
                    about = dram.tile([NCORES * H, 72], fp32, tag="about",
                                      addr_space="Shared", name="about")
                    nc.sync.dma_start(abin[:],
                                      staga[:].rearrange("p a g s -> p (a g s)"))
                    nc.gpsimd.collective_compute(
                        "AllGather", ALU.bypass, replica_groups=RG,
                        ins=[abin[:]], outs=[about[:]],
                    )
                    ag8 = loop.tile([H, NCORES, 72], fp32, tag="ag8",
                                    name="ag8")
                    nc.sync.dma_start(ag8[:],
                                      about[:].rearrange("(r p) s -> p r s",
                                                         r=NCORES))
                    for k in range(12):
                        nc.vector.bn_aggr(mv12[:, k, :],
                                          ag8[:, :, 6 * k:6 * (k + 1)])
                sd12 = loop.tile([H, 12], fp32, tag="sd12", name="sd12")
                nc.scalar.activation(sd12[:], mv12[:, :, 1], AF.Sqrt,
                                     bias=eps_t[:])
                r12 = loop.tile([H, 12], fp32, tag="r12", name="r12")
                nc.vector.reciprocal(r12[:], sd12[:])
                S12 = loop.tile([H, 12], fp32, tag="S12", name="S12")
                nc.gpsimd.tensor_tensor(S12[:], r12[:], P["gcat12"][:],
                                        op=ALU.mult)
                TM12 = loop.tile([H, 12], fp32, tag="TM12", name="TM12")
                nc.gpsimd.tensor_tensor(TM12[:], mv12[:, :, 0], S12[:],
                                        op=ALU.mult)
                SH12 = loop.tile([H, 12], fp32, tag="SH12", name="SH12")
                nc.gpsimd.tensor_tensor(SH12[:], P["bcat12"][:], TM12[:],
                                        op=ALU.subtract)
                stagb = loop.tile([H, 2, 6], fp32, tag="stagb", bufs=2,
                                  name="stagb")
                if t == 0:
                    nc.vector.memset(stagb[:, 1, :], 0.0)
                if t == T:
                    nc.vector.memset(stagb[:, 0, :], 0.0)
                ga0 = ga1 = None
                c0n = c1n = None
                if has0:
                    u0p = loop.tile([H, G], fp32, tag="u0p", name="u0p")
                    nc.gpsimd.tensor_tensor(u0p[:], ugam0_all[:, t, :],
                                            sd12[:, 0:4], op=ALU.mult)
                    v0p = loop.tile([H, G], fp32, tag="v0p", name="v0p")
                    nc.gpsimd.tensor_tensor(v0p[:], SH12[:, 0:4],
                                            vih0_all[:, t, :], op=ALU.add)
                    for q in range(G):
                        nc.vector.scalar_tensor_tensor(
                            pw0[:, q, :], in0=wi_t[:, q, :],
                            scalar=u0p[:, q:q + 1], in1=pw0[:, q, :],
                            op0=ALU.mult, op1=ALU.add)
                    ga0 = loop.tile([H, G, BL], fp32, tag="ga0", bufs=2,
                                    name="ga0")
                    for q, fn in enumerate((AF.Sigmoid, AF.Sigmoid,
                                            AF.Sigmoid, AF.Tanh)):
                        nc.scalar.activation(ga0[:, q, :], pw0[:, q, :], fn,
                                             bias=v0p[:, q:q + 1],
                                             scale=S12[:, q:q + 1])
                    t10 = loop.tile([H, BL], fp32, tag="t10", name="t10")
                    nc.vector.tensor_tensor(t10[:], ga0[:, 1, :], ga0[:, 3, :],
                                            op=ALU.mult)
                    t20 = loop.tile([H, BL], fp32, tag="t20", name="t20")
                    nc.gpsimd.tensor_tensor(t20[:], ga0[:, 0, :], c0[:],
                                            op=ALU.mult)
                    c0n = loop.tile([H, BL], fp32, tag="c0", name="c0n")
                    nc.vector.tensor_tensor(c0n[:], t10[:], t20[:], op=ALU.add)
                    nc.vector.bn_stats(stagb[:, 0, :], c0n[:])
                if has1:
                    tu1 = loop.tile([H, G], fp32, tag="tu1", name="tu1")
                    nc.gpsimd.tensor_tensor(tu1[:], P["ratio1"][:],
                                            r12[:, 8:12], op=ALU.mult)
                    u1p = loop.tile([H, G], fp32, tag="u1p", name="u1p")
                    nc.gpsimd.tensor_tensor(u1p[:], tu1[:], sd12[:, 4:8],
                                            op=ALU.mult)
                    v1p = loop.tile([H, G], fp32, tag="v1p", name="v1p")
                    nc.gpsimd.tensor_tensor(v1p[:], SH12[:, 4:8], SH12[:, 8:12],
                                            op=ALU.add)
                    for q in range(G):
                        nc.vector.scalar_tensor_tensor(
                            pw1[:, q, :], in0=wi1sb[:, q, :],
                            scalar=u1p[:, q:q + 1], in1=pw1[:, q, :],
                            op0=ALU.mult, op1=ALU.add)
                    ga1 = loop.tile([H, G, BL], fp32, tag="ga1", bufs=2,
                                    name="ga1")
                    for q, fn in enumerate((AF.Sigmoid, AF.Sigmoid,
                                            AF.Sigmoid, AF.Tanh)):
                        nc.scalar.activation(ga1[:, q, :], pw1[:, q, :], fn,
                                             bias=v1p[:, q:q + 1],
                                             scale=S12[:, 4 + q:5 + q])
                    t11 = loop.tile([H, BL], fp32, tag="t11", name="t11")
                    nc.vector.tensor_tensor(t11[:], ga1[:, 1, :], ga1[:, 3, :],
                                            op=ALU.mult)
                    t21 = loop.tile([H, BL], fp32, tag="t21", name="t21")
                    nc.gpsimd.tensor_tensor(t21[:], ga1[:, 0, :], c1[:],
                                            op=ALU.mult)
                    c1n = loop.tile([H, BL], fp32, tag="c1", name="c1n")
                    nc.vector.tensor_tensor(c1n[:], t11[:], t21[:], op=ALU.add)
                    nc.vector.bn_stats(stagb[:, 1, :], c1n[:])
                # ---- AGb: {c0, c1} stats ----
                mvc2 = loop.tile([H, 2, 2], fp32, tag="mvc2", name="mvc2")
                if local_stats:
                    for k in range(2):
                        nc.vector.bn_aggr(mvc2[:, k, :], stagb[:, k, :])
                else:
                    bbin = dram.tile([H, 12], fp32, tag="bbin", name="bbin")
                    bbout = dram.tile([NCORES * H, 12], fp32, tag="bbout",
                                      addr_space="Shared", name="bbout")
                    nc.sync.dma_start(bbin[:],
                                      stagb[:].rearrange("p a s -> p (a s)"))
                    nc.gpsimd.collective_compute(
                        "AllGather", ALU.bypass, replica_groups=RG,
                        ins=[bbin[:]], outs=[bbout[:]],
                    )
                    bg8 = loop.tile([H, NCORES, 12], fp32, tag="bg8",
                                    name="bg8")
                    nc.sync.dma_start(bg8[:],
                                      bbout[:].rearrange("(r p) s -> p r s",
                                                         r=NCORES))
                    for k in range(2):
                        nc.vector.bn_aggr(mvc2[:, k, :],
                                          bg8[:, :, 6 * k:6 * (k + 1)])
                sdc2 = loop.tile([H, 2], fp32, tag="sdc2", name="sdc2")
                nc.scalar.activation(sdc2[:], mvc2[:, :, 1], AF.Sqrt,
                                     bias=eps_t[:])
                rc2 = loop.tile([H, 2], fp32, tag="rc2", name="rc2")
                nc.vector.reciprocal(rc2[:], sdc2[:])
                scc = loop.tile([H, 2], fp32, tag="scc", name="scc")
                nc.gpsimd.tensor_tensor(scc[:], rc2[:], P["gc2"][:],
                                        op=ALU.mult)
                tmc = loop.tile([H, 2], fp32, tag="tmc", name="tmc")
                nc.gpsimd.tensor_tensor(tmc[:], mvc2[:, :, 0], scc[:],
                                        op=ALU.mult)
                shc = loop.tile([H, 2], fp32, tag="shc", name="shc")
                nc.gpsimd.tensor_tensor(shc[:], P["bc2"][:], tmc[:],
                                        op=ALU.subtract)
                if has0:
                    tn0 = loop.tile([H, BL], fp32, tag="tn0", name="tn0")
                    nc.scalar.activation(tn0[:], c0n[:], AF.Tanh,
                                         bias=shc[:, 0:1], scale=scc[:, 0:1])
                    h0n = loop.tile([H, BL], fp32, tag="h0", name="h0n")
                    nc.gpsimd.tensor_tensor(h0n[:], ga0[:, 2, :], tn0[:],
                                            op=ALU.mult)
                    # produce wi1[t] for layer 1 (consumed next step)
                    pwm = psum.tile([H, G, BL], fp32, tag="w1", bufs=1,
                                    name="pwm")
                    for q in range(G):
                        nc.tensor.matmul(pwm[:, q, :],
                                         wih1_sb[:, q * H:(q + 1) * H],
                                         h0n[:], start=True, stop=True)
                    staga_n = loop.tile([H, 3, G, 6], fp32, tag="staga",
                                        bufs=2, name="staga_n")
                    for q in range(G):
                        nc.vector.bn_stats(staga_n[:, 2, q, :], pwm[:, q, :])
                    wi1n = loop.tile([H, G, BL], fp32, tag="wi1sb", bufs=2,
                                     name="wi1n")
                    nc.scalar.copy(wi1n[:], pwm[:])
                    wi1sb = wi1n
                    staga = staga_n
                    h0 = h0n
                    c0 = c0n
                if has1:
                    tn1 = loop.tile([H, BL], fp32, tag="tn1", name="tn1")
                    nc.scalar.activation(tn1[:], c1n[:], AF.Tanh,
                                         bias=shc[:, 1:2], scale=scc[:, 1:2])
                    h1n = loop.tile([H, BL], fp32, tag="h1", name="h1n")
                    nc.gpsimd.tensor_tensor(h1n[:], ga1[:, 2, :], tn1[:],
                                            op=ALU.mult)
                    h1 = h1n
                    c1 = c1n
            return h1

        h_fin = None
        for _rep in range(repeats):
            if not skip_phase0:
                phase0()
                aggregate0()
            if not skip_mainloop:
                h_fin = mainloop()
        if h_fin is None:
            h_fin = loop.tile([H, BL], fp32, tag="h1", name="hf")
            nc.vector.memset(h_fin[:], 0.0)

        for ci in range(2):
            pf = psum.tile([H, O], fp32, tag="w1", name="pf")
            nc.tensor.matmul(pf[:], h_fin[:, ci * H:(ci + 1) * H], fcwT_sb[:],
                             start=True, stop=True)
            yo = loop.tile([H, O], fp32, tag="yo", name="yo")
            nc.scalar.copy(yo[:], pf[:])
            nc.sync.dma_start(
                y[:].rearrange("(c p) o -> c p o", c=2)[ci], yo[:])

    nc.compile()
    return nc


VERSION = 2

_NC_CACHE = None


def _get_nc():
    global _NC_CACHE
    if _NC_CACHE is None:
        _NC_CACHE = _build_v2() if VERSION == 2 else _build()
    return _NC_CACHE


def _prep_inputs(sequences, w_ih0, w_hh0, b0, g_ih0, be_ih0, g_hh0, be_hh0,
                 g_c0, be_c0, w_ih1, w_hh1, b1, g_ih1, be_ih1, g_hh1, be_hh1,
                 g_c1, be_c1, fc_w, fc_b):
    f32 = np.float32

    def pg(v):  # (512,) -> (128, 4)
        return np.ascontiguousarray(np.asarray(v, f32).reshape(G, H).T)

    common = {
        "wih0": np.ascontiguousarray(np.asarray(w_ih0, f32)),
        "whh0": np.ascontiguousarray(np.asarray(w_hh0, f32)),
        "wih1": np.ascontiguousarray(np.asarray(w_ih1, f32)),
        "whh1": np.ascontiguousarray(np.asarray(w_hh1, f32)),
        "fcwT": np.ascontiguousarray(np.asarray(fc_w, f32).T),
    }
    if VERSION == 2:
        common.update({
            "gih0": pg(g_ih0),
            "beihb0": pg(np.asarray(be_ih0) + np.asarray(b0)),
            "ratio0": pg(np.asarray(g_ih0) / np.asarray(g_hh0)),
            "ratio1": pg(np.asarray(g_ih1) / np.asarray(g_hh1)),
            "gcat12": np.concatenate([pg(g_hh0), pg(g_hh1), pg(g_ih1)],
                                     axis=1),
            "bcat12": np.concatenate(
                [pg(be_hh0), pg(be_hh1),
                 pg(np.asarray(be_ih1) + np.asarray(b1))], axis=1),
            "gc2": np.stack([np.asarray(g_c0, f32),
                             np.asarray(g_c1, f32)], axis=1).copy(),
            "bc2": np.stack([np.asarray(be_c0, f32),
                             np.asarray(be_c1, f32)], axis=1).copy(),
        })
    else:
        common.update({
            "gih0": pg(g_ih0),
            "beihb0": pg(np.asarray(be_ih0) + np.asarray(b0)),
            "ghh0": pg(g_hh0), "behh0": pg(be_hh0),
            "gc0": np.asarray(g_c0, f32).reshape(H, 1).copy(),
            "bec0": np.asarray(be_c0, f32).reshape(H, 1).copy(),
            "gih1": pg(g_ih1),
            "beihb1": pg(np.asarray(be_ih1) + np.asarray(b1)),
            "ghh1": pg(g_hh1), "behh1": pg(be_hh1),
            "gc1": np.asarray(g_c1, f32).reshape(H, 1).copy(),
            "bec1": np.asarray(be_c1, f32).reshape(H, 1).copy(),
        })
    seq = np.asarray(sequences, f32)
    in_maps = []
    for c in range(NCORES):
        m = dict(common)
        m["xT"] = np.ascontiguousarray(
            seq[c * BL:(c + 1) * BL].transpose(2, 1, 0))  # (I, T, BL)
        in_maps.append(m)
    return in_maps


def kernel(**inputs):
    nc = _get_nc()
    in_maps = _prep_inputs(**inputs)
    last_exc = None
    for attempt in range(3):
        try:
            res = run_bass_kernel_spmd(nc, in_maps,
                                       core_ids=list(range(NCORES)),
                                       trace=False)
            break
        except Exception as e:  # transient runtime INTERNAL errors observed
            last_exc = e
            time.sleep(5.0 * (attempt + 1))
    else:
        raise last_exc
    ys = [res.results[c]["y"] for c in range(NCORES)]
    out = np.concatenate(ys, axis=0)  # (B, O)
    out = out + np.asarray(inputs["fc_b"], np.float32)[None, :]
    return out.astype(np.float32)
